# revision 71
# baseline (speedup 1.0000x reference)
"""Conformer block (macaron FF + RMLA attention + gated depthwise conv) on
8 Trainium2 NeuronCores, data-parallel over batch (B=8 -> 1 seq/core).

The residual stream lives channel-major ([D, T]) in SBUF for the whole
kernel. LayerNorm is applied by standardizing the input once into a
quantized "xhat" tile (per-token mean/invstd come from ones-matmul
statistics + K=1 broadcast matmuls); gammas/betas are folded into the
adjacent weights/biases on the host. Projections run in fp8e4 with the
DoubleRow perf mode (2 k-chunks per matmul, 2x PE throughput); weights are
pre-scaled by a power of two into fp8's normal range and descaled for free
via the eviction activation's scale argument. The depthwise conv runs as
fp8 DoubleRow diagonal matmuls over shifted windows. Attention scores/probs
stay bf16 (softmax denominators via a ones-column appended to V).
"""
import os
from contextlib import ExitStack

import numpy as np
import ml_dtypes

import concourse.bacc as bacc
import concourse.tile as tile
import concourse.mybir as mybir
from concourse.bass_utils import run_bass_kernel_spmd

B, T, D = 8, 1024, 1024
H, HD, KVH, R = 16, 64, 4, 256
KW = 31
FF = 4 * D
EPS = 1e-5
P = 128
DC = D // P            # 8 residual chunks
FFC = FF // P          # 32
RC = R // P            # 2
N_CORES = 8

dt = mybir.dt
Alu = mybir.AluOpType
Act = mybir.ActivationFunctionType
DR = mybir.MatmulPerfMode.DoubleRow

bf16 = ml_dtypes.bfloat16
f8e4 = ml_dtypes.float8_e4m3fn

PHASES = int(os.environ.get("BASS_PHASES", "5"))
DEBUG = int(os.environ.get("BASS_DEBUG", "0"))
# fp8 spots (DoubleRow). ff1/ff2 toggle both matmuls of that FF block.
FP8 = set(os.environ.get(
    "BASS_FP8", "q,kva,kvb,wo,pw1,pw2,dw,ff1,ff2").split(","))

SX = 32.0              # activation pre-scale for fp8 xhat/h1/glu
SO = 8.0               # activation pre-scale for fp8 attention outputs


# ---------------------------------------------------------------- host prep

def _pow2_scale(w):
    m = float(np.abs(w).max())
    if m == 0.0:
        return 1.0
    return float(2.0 ** np.floor(np.log2(240.0 / m)))


def _shuffle_w(W):
    """[Kd, Nd] -> [NC, P, Kd] slab n: tile[:, kc*128:(kc+1)*128] =
    W[kc-chunk, n-chunk] (stationary for output chunk n, bf16 path)."""
    Kd, Nd = W.shape
    KC, NC = Kd // P, Nd // P
    arr = W.reshape(KC, P, NC, P).transpose(2, 1, 0, 3).reshape(NC, P, Kd)
    return np.ascontiguousarray(arr)


def _cols(v):
    """[N] bias -> [P, N/128] column tile (col n = bias of chunk n)."""
    return np.ascontiguousarray(v.reshape(-1, P).T)


def prep_inputs(inputs):
    f32 = np.float32
    g = {}
    meta = {}

    def W(name):
        return np.asarray(inputs[name], f32)

    def proj_w(key, wf, spot, sx):
        """Pack a [K, N] stationary; fp8 spots get DoubleRow layout
        [NC, P, KC//2, 2, P] with a pow2 scale; returns eviction descale."""
        if spot in FP8:
            sw = _pow2_scale(wf)
            arr = _shuffle_w(wf * sw)          # [NC, P, K]
            NC = arr.shape[0]
            KC = wf.shape[0] // P
            arr = arr.reshape(NC, P, KC // 2, 2, P)
            g[key] = np.ascontiguousarray(arr).astype(f8e4)
            return 1.0 / (sw * sx)
        g[key] = _shuffle_w(wf).astype(bf16)
        return 1.0

    # ff1 (LN gamma folded; 0.5 residual scale folded into w2/b2)
    ff1_on = 'ff1' in FP8
    meta['d_w1a'] = proj_w('w1a', W('ff1_ng')[:, None] * W('ff1_w1'),
                           'ff1', SX if ff1_on else 1.0)
    g['c1a'] = _cols(W('ff1_nb') @ W('ff1_w1') + W('ff1_b1'))
    meta['d_w2a'] = proj_w('w2a', 0.5 * W('ff1_w2'), 'ff1',
                           SX if ff1_on else 1.0)
    g['c2a'] = _cols(0.5 * W('ff1_b2'))
    # attention projections
    meta['d_wqa'] = proj_w('wqa', W('attn_ng')[:, None] * W('wq'), 'q', SX)
    g['cqa'] = _cols(W('attn_nb') @ W('wq'))
    meta['d_wkvaa'] = proj_w(
        'wkvaa', (W('attn_ng')[:, None] * W('wkva'))[:, :R], 'kva', SX)
    g['ckvaa'] = _cols((W('attn_nb') @ W('wkva'))[:R])
    meta['d_wkvba'] = proj_w('wkvba', W('kvn_g')[:, None] * W('wkvb'),
                             'kvb', SX)
    g['ckvba'] = _cols(W('kvn_b') @ W('wkvb'))
    meta['d_woa'] = proj_w('woa', W('wo'), 'wo', SO)
    # conv module
    meta['d_wp1a'] = proj_w('wp1a', W('conv_ng')[:, None] * W('pw1_w'),
                            'pw1', SX)
    g['cp1a'] = _cols(W('conv_nb') @ W('pw1_w') + W('pw1_b'))
    sbn = W('bn_g') / np.sqrt(W('bn_rv') + EPS)
    g['tbna'] = _cols((W('dw_b') - W('bn_rm')) * sbn + W('bn_b'))
    meta['d_wp2a'] = proj_w('wp2a', W('pw2_w'), 'pw2', 1.0)
    g['cp2a'] = _cols(W('pw2_b'))
    # depthwise conv as DoubleRow diag: pairs (tap j, tap j+16), j=0..15;
    # tap 31 zero-padded. dwdiag[c, p, j, t, col] = dwf[c*P+col, j+16*t]*(p==col)
    dwf = np.asarray(inputs['dw_w'], f32)[:, 0, :] * sbn[:, None]   # [D, 31]
    if 'dw' in FP8:
        sdw = _pow2_scale(dwf)
        dwp = np.concatenate([dwf * sdw, np.zeros((D, 1), f32)], 1)  # [D, 32]
        diag = np.zeros((DC, P, 16, 2, P), f32)
        idx = np.arange(P)
        for c in range(DC):
            for j in range(16):
                for t_ in range(2):
                    diag[c, idx, j, t_, idx] = dwp[c * P:(c + 1) * P, j + 16 * t_]
        g['dwdiag'] = diag.astype(f8e4)
        meta['d_dw'] = 1.0 / (sdw * SX)
    else:
        # bf16 fallback: 21 PE taps + 10 vector taps (baseline scheme)
        SPLIT = 21
        diag = np.zeros((DC, P, SPLIT, P), f32)
        idx = np.arange(P)
        for c in range(DC):
            for j in range(SPLIT):
                diag[c, idx, j, idx] = dwf[c * P:(c + 1) * P, j]
        g['dwdiag'] = diag.reshape(DC, P, SPLIT * P).astype(bf16)
        g['dwcol'] = np.ascontiguousarray(
            dwf.reshape(DC, P, KW).transpose(1, 0, 2).reshape(P, DC * KW))
        meta['d_dw'] = 1.0
    # ff2
    ff2_on = 'ff2' in FP8
    meta['d_w1b'] = proj_w('w1b', W('ff2_ng')[:, None] * W('ff2_w1'),
                           'ff2', SX if ff2_on else 1.0)
    g['c1b'] = _cols(W('ff2_nb') @ W('ff2_w1') + W('ff2_b1'))
    meta['d_w2b'] = proj_w('w2b', 0.5 * W('ff2_w2'), 'ff2',
                           SX if ff2_on else 1.0)
    g['c2b'] = _cols(0.5 * W('ff2_b2'))
    # final LN affine
    g['finga'] = _cols(W('fin_g'))
    g['finba'] = _cols(W('fin_b'))
    # rope tables (transposed, tiled x2 heads per 128 partitions)
    inv = 1.0 / (10000.0 ** (np.arange(0, HD, 2, dtype=f32) / HD))
    t = np.arange(T, dtype=f32)
    fr = np.einsum('i,j->ij', t, inv)
    emb = np.concatenate([fr, fr], -1)                        # [T, 64]
    cosT = np.cos(emb).T.astype(f32)                          # [64, T]
    sinT = np.sin(emb).T.astype(f32)
    g['cos2'] = np.ascontiguousarray(
        np.concatenate([cosT, cosT], 0)).astype(bf16)
    g['sin2'] = np.ascontiguousarray(
        np.concatenate([sinT, sinT], 0)).astype(bf16)
    p2 = np.zeros((P, P), f32)
    for b in range(2):
        o = 64 * b
        for d_ in range(32):
            p2[o + 32 + d_, o + d_] = -1.0
            p2[o + d_, o + 32 + d_] = 1.0
    g['p2m'] = p2.astype(bf16)
    g['ident'] = np.eye(P, dtype=f32).astype(bf16)
    g['ones1'] = np.ones((1, P), f32)
    sel2 = np.zeros((2, P), f32)
    sel2[0, 0:64] = 1.0
    sel2[1, 64:P] = 1.0
    g['sel2'] = sel2
    g['onesp'] = np.ones((P, 1), f32)
    g['onespb'] = np.ones((P, 1), f32).astype(bf16)
    return g, meta


# ------------------------------------------------------------- device build

def build(meta):
    nc = bacc.Bacc("TRN2", target_bir_lowering=False, debug=False,
                   enable_asserts=False, num_devices=N_CORES)
    f32, f32r, b16, f8 = dt.float32, dt.float32r, dt.bfloat16, dt.float8e4

    def wdt(spot):
        return f8 if spot in FP8 else b16

    def din(name, shape, d):
        return nc.dram_tensor(name, shape, d, kind="ExternalInput").ap()

    def wshape(spot, KC, NC):
        if spot in FP8:
            return (NC, P, KC // 2, 2, P)
        return (NC, P, KC * P)

    xT = din('xT', (D, T), f32r)
    w1a = din('w1a', wshape('ff1', DC, FFC), wdt('ff1'))
    c1a = din('c1a', (P, FFC), f32)
    w2a = din('w2a', wshape('ff1', FFC, DC), wdt('ff1'))
    c2a = din('c2a', (P, DC), f32)
    wqa = din('wqa', wshape('q', DC, 8), wdt('q'))
    cqa = din('cqa', (P, 8), f32)
    wkvaa = din('wkvaa', wshape('kva', DC, RC), wdt('kva'))
    ckvaa = din('ckvaa', (P, RC), f32)
    wkvba = din('wkvba', wshape('kvb', RC, 4), wdt('kvb'))
    ckvba = din('ckvba', (P, 4), f32)
    woa = din('woa', wshape('wo', DC, DC), wdt('wo'))
    wp1a = din('wp1a', wshape('pw1', DC, 16), wdt('pw1'))
    cp1a = din('cp1a', (P, 16), f32)
    tbna = din('tbna', (P, DC), f32)
    wp2a = din('wp2a', wshape('pw2', DC, DC), wdt('pw2'))
    cp2a = din('cp2a', (P, DC), f32)
    if 'dw' in FP8:
        dwdiag = din('dwdiag', (DC, P, 16, 2, P), f8)
        dwcold = None
    else:
        dwdiag = din('dwdiag', (DC, P, 21 * P), b16)
        dwcold = din('dwcol', (P, DC * KW), f32)
    w1b = din('w1b', wshape('ff2', DC, FFC), wdt('ff2'))
    c1b = din('c1b', (P, FFC), f32)
    w2b = din('w2b', wshape('ff2', FFC, DC), wdt('ff2'))
    c2b = din('c2b', (P, DC), f32)
    finga = din('finga', (P, DC), f32)
    finba = din('finba', (P, DC), f32)
    cos2d = din('cos2', (P, T), b16)
    sin2d = din('sin2', (P, T), b16)
    p2md = din('p2m', (P, P), b16)
    identd = din('ident', (P, P), b16)
    ones1d = din('ones1', (1, P), f32r)
    sel2d = din('sel2', (2, P), f32r)
    onespd = din('onesp', (P, 1), f32r)
    onespbd = din('onespb', (P, 1), b16)

    outT = nc.dram_tensor('outT', (D, T), f32r, kind="ExternalOutput").ap()

    def ddram(name, shape, d):
        return nc.dram_tensor(name, shape, d, kind="ExternalOutput").ap()

    with tile.TileContext(nc) as tc, ExitStack() as top:
        cpool = top.enter_context(tc.tile_pool(name="const", bufs=1))
        res_pool = top.enter_context(tc.tile_pool(name="res", bufs=1))
        xh_pool = top.enter_context(tc.tile_pool(name="xh", bufs=1))

        def ctile(src, shape, d, name):
            t_ = cpool.tile(shape, d, name=name)
            nc.sync.dma_start(t_[:], src[:])
            return t_

        c1t = ctile(c1a, [P, FFC], f32, "c1t")
        c2t = ctile(c2a, [P, DC], f32, "c2t")
        cqt = ctile(cqa, [P, 8], f32, "cqt")
        ckvat = ctile(ckvaa, [P, RC], f32, "ckvat")
        ckvbt = ctile(ckvba, [P, 4], f32, "ckvbt")
        cp1t = ctile(cp1a, [P, 16], f32, "cp1t")
        tbnt = ctile(tbna, [P, DC], f32, "tbnt")
        cp2t = ctile(cp2a, [P, DC], f32, "cp2t")
        c1bt = ctile(c1b, [P, FFC], f32, "c1bt")
        c2bt = ctile(c2b, [P, DC], f32, "c2bt")
        fingt = ctile(finga, [P, DC], f32, "fingt")
        finbt = ctile(finba, [P, DC], f32, "finbt")
        cos2t = ctile(cos2d, [P, T], b16, "cos2t")
        sin2t = ctile(sin2d, [P, T], b16, "sin2t")
        p2mt = ctile(p2md, [P, P], b16, "p2mt")
        identt = ctile(identd, [P, P], b16, "identt")
        ones1t = ctile(ones1d, [1, P], f32r, "ones1t")
        sel2t = ctile(sel2d, [2, P], f32r, "sel2t")
        onespt = ctile(onespd, [P, 1], f32r, "onespt")
        onespbt = ctile(onespbd, [P, 1], b16, "onespbt")
        dwcolt = (ctile(dwcold, [P, DC * KW], f32, "dwcolt")
                  if dwcold is not None else None)
        epst = cpool.tile([P, 1], dt.float32, name="epst")
        nc.gpsimd.memset(epst[:], EPS)

        # ------- cross-phase LN stats: accumulated in the producing -------
        # ------- phase's eviction tail, consumed at the next phase -------
        class NextStats:
            """Per-token LN stats over the residual, accumulated chunk-by-
            chunk as the previous phase finalizes each res chunk."""

            def __init__(self, tag, dred):
                self.tag, self.dred = tag, dred
                self.ctx = ExitStack()
                self.opened = False

            def _open(self):
                self.lnp = self.ctx.enter_context(
                    tc.tile_pool(name=f"lnp_{self.tag}", bufs=2,
                                 space="PSUM", side="right"))
                self.lns = self.ctx.enter_context(
                    tc.tile_pool(name=f"lns_{self.tag}", bufs=1,
                                 side="right"))
                self.s1 = self.lnp.tile([1, T], dt.float32, tag="lnps",
                                        name=f"s1_{self.tag}")
                self.s2 = self.lnp.tile([1, T], dt.float32, tag="lnps",
                                        name=f"s2_{self.tag}")
                self.opened = True

            def hook(self, c, nch=DC):
                assert self.opened, f"stats {self.tag} pools not opened"
                s_ = self.lns.tile([P, T], dt.float32r, tag="sq", bufs=2,
                                   name=f"sq_{self.tag}{c}")
                nc.scalar.square(s_[:], res[c].bitcast(dt.float32))
                for h in range(2):
                    sl = slice(h * 512, (h + 1) * 512)
                    nc.tensor.matmul(self.s1[:, sl], onespt[:],
                                     res[c][:, sl],
                                     start=(c == 0), stop=(c == nch - 1))
                for h in range(2):
                    sl = slice(h * 512, (h + 1) * 512)
                    nc.tensor.matmul(self.s2[:, sl], onespt[:], s_[:, sl],
                                     start=(c == 0), stop=(c == nch - 1))

            def finalize(self, sx):
                f32 = dt.float32
                tag = self.tag
                lns = self.lns
                m_t = lns.tile([1, T], dt.float32r, name=f"m_{tag}")
                a_t = lns.tile([1, T], dt.float32r, name=f"a_{tag}")
                nc.vector.tensor_scalar(m_t[:], self.s1[:], 1.0 / self.dred,
                                        None, Alu.mult)
                ms = lns.tile([1, T], f32, name=f"ms_{tag}")
                nc.vector.tensor_tensor(ms[:], m_t.bitcast(f32)[:],
                                        m_t.bitcast(f32)[:], Alu.mult)
                v_ = lns.tile([1, T], f32, name=f"v_{tag}")
                nc.vector.scalar_tensor_tensor(v_[:], self.s2[:],
                                               1.0 / self.dred, ms[:],
                                               Alu.mult, Alu.subtract)
                sd = lns.tile([1, T], f32, name=f"sd_{tag}")
                nc.scalar.activation(sd[:], v_[:], Act.Sqrt,
                                     bias=epst[0:1, 0:1])
                af = lns.tile([1, T], f32, name=f"af_{tag}")
                nc.vector.reciprocal_approx_fast(out=af[:], in_=sd[:])
                if sx != 1.0:
                    nc.vector.tensor_scalar(a_t[:], af[:], sx, None,
                                            Alu.mult)
                else:
                    nc.vector.tensor_copy(a_t[:], af[:])
                return m_t, a_t

            def broadcast(self, m_t, a_t):
                """Returns (mb_sbuf, ab_psum): gpsimd can't read PSUM."""
                f32 = dt.float32
                mb = self.lnp.tile([P, T], f32, tag="lnps",
                                   name=f"mb_{self.tag}")
                ab = self.lnp.tile([P, T], f32, tag="lnps",
                                   name=f"ab_{self.tag}")
                for h in range(2):
                    sl = slice(h * 512, (h + 1) * 512)
                    nc.tensor.matmul(mb[:, sl], ones1t[:], m_t[:, sl],
                                     start=True, stop=True)
                    nc.tensor.matmul(ab[:, sl], ones1t[:], a_t[:, sl],
                                     start=True, stop=True)
                return mb, ab

            def consume(self, dst3, sx, nch=DC):
                """Standardize res into dst3 (xhat pipeline: gpsimd sub,
                vector mult+cast); closes the stats pools. Must be called
                before the consuming phase opens any of its own pools."""
                m_t, a_t = self.finalize(sx)
                mb, ab = self.broadcast(m_t, a_t)
                f32 = dt.float32
                tm_p = self.ctx.enter_context(
                    tc.tile_pool(name=f"lntm_{self.tag}", bufs=3,
                                 side="right"))
                # emit h0 for all chunks first so the first projection
                # matmuls (which sweep every chunk of one half) start at
                # half-time
                tms = [tm_p.tile([P, T], f32, tag="lntmp", bufs=3,
                                 name=f"lntmp_{self.tag}{c}")
                       for c in range(3)]
                for h in range(2):
                    sl = slice(h * 512, (h + 1) * 512)
                    for c in range(nch):
                        tm = tms[c % 3]
                        nc.vector.tensor_tensor(tm[:, sl],
                                                res[c].bitcast(f32)[:, sl],
                                                mb[:, sl], Alu.subtract)
                        nc.vector.tensor_tensor(dst3[:, c, sl], tm[:, sl],
                                                ab[:, sl], Alu.mult)
                self.ctx.close()

        st_ff1 = NextStats("ff1", D)
        st_at = NextStats("at", D) if PHASES >= 2 else None
        st_cv = NextStats("cv", D) if PHASES >= 3 else None
        st_f2 = NextStats("ff2", D) if PHASES >= 4 else None
        st_fin = NextStats("fin", D) if PHASES >= 5 else None

        st_ff1._open()
        res = []
        for c in range(DC):
            r_ = res_pool.tile([P, T], f32r, name=f"res{c}")
            nc.sync.dma_start(r_[:], xT[c * P:(c + 1) * P, :])
            res.append(r_)
        for c in range(DC):
            st_ff1.hook(c)

        # ------------- LN: stats + standardized (scaled) xhat -------------
        def ln_stats(ctx, tag, src_tiles, nch, dred, sx):
            """Per-token mean/scaled-invstd of src over nch*128 channels."""
            src_is_b16 = src_tiles[0].dtype == b16
            ones_stat = onespbt if src_is_b16 else onespt

            def rd(ap):
                return ap if src_is_b16 else ap.bitcast(f32)

            lnp = ctx.enter_context(
                tc.tile_pool(name=f"lnp_{tag}", bufs=2, space="PSUM",
                             side="right"))
            lns = ctx.enter_context(tc.tile_pool(name=f"lns_{tag}", bufs=1,
                                                 side="right"))
            sq = []
            for c in range(nch):
                s_ = lns.tile([P, T], f32r, tag="sq", bufs=2,
                              name=f"sq_{tag}{c}")
                nc.scalar.square(s_[:], rd(src_tiles[c][:]))
                sq.append(s_)
            s1 = lnp.tile([1, T], f32, tag="lnps", name=f"s1_{tag}")
            s2 = lnp.tile([1, T], f32, tag="lnps", name=f"s2_{tag}")
            for c in range(nch):
                for h in range(2):
                    sl = slice(h * 512, (h + 1) * 512)
                    nc.tensor.matmul(s1[:, sl], ones_stat[:],
                                     src_tiles[c][:, sl],
                                     start=(c == 0), stop=(c == nch - 1))
            for c in range(nch):
                for h in range(2):
                    sl = slice(h * 512, (h + 1) * 512)
                    nc.tensor.matmul(s2[:, sl], onespt[:], sq[c][:, sl],
                                     start=(c == 0), stop=(c == nch - 1))
            m_t = lns.tile([1, T], f32r, name=f"m_{tag}")
            a_t = lns.tile([1, T], f32r, name=f"a_{tag}")
            nc.vector.tensor_scalar(m_t[:], s1[:], 1.0 / dred, None, Alu.mult)
            ms = lns.tile([1, T], f32, name=f"ms_{tag}")
            nc.vector.tensor_tensor(ms[:], m_t.bitcast(f32)[:],
                                    m_t.bitcast(f32)[:], Alu.mult)
            v_ = lns.tile([1, T], f32, name=f"v_{tag}")
            nc.vector.scalar_tensor_tensor(v_[:], s2[:], 1.0 / dred, ms[:],
                                           Alu.mult, Alu.subtract)
            sd = lns.tile([1, T], f32, name=f"sd_{tag}")
            nc.scalar.activation(sd[:], v_[:], Act.Sqrt, bias=epst[0:1, 0:1])
            af = lns.tile([1, T], f32, name=f"af_{tag}")
            nc.vector.reciprocal_approx_fast(out=af[:], in_=sd[:])
            if sx != 1.0:
                nc.vector.tensor_scalar(a_t[:], af[:], sx, None, Alu.mult)
            else:
                nc.vector.tensor_copy(a_t[:], af[:])
            return m_t, a_t, lnp, rd

        def ln_apply(ctx, tag, src_tiles, nch, dst3, m_t, a_t, lnp, rd):
            """dst3[:, c, :] = (src_c - mean)*scaled_invstd per token."""
            mb = lnp.tile([P, T], f32, tag="lnps", name=f"mb_{tag}")
            ab = lnp.tile([P, T], f32, tag="lnps", name=f"ab_{tag}")
            for h in range(2):
                sl = slice(h * 512, (h + 1) * 512)
                nc.tensor.matmul(mb[:, sl], ones1t[:], m_t[:, sl],
                                 start=True, stop=True)
                nc.tensor.matmul(ab[:, sl], ones1t[:], a_t[:, sl],
                                 start=True, stop=True)
            tm_p = ctx.enter_context(tc.tile_pool(name=f"lntm_{tag}", bufs=2,
                                                  side="right"))
            for c in range(nch):
                tm = tm_p.tile([P, T], f32, tag="lntmp", bufs=2,
                               name=f"lntmp_{tag}{c}")
                nc.vector.tensor_tensor(tm[:], rd(src_tiles[c][:]), mb[:],
                                        Alu.subtract)
                nc.vector.tensor_tensor(dst3[:, c, :], tm[:], ab[:], Alu.mult)

        # ------------- projection: fp8 DoubleRow or bf16 -------------
        def proj(pp, wt, x3, KC, spot, nm, evict):
            """psum[:, h*512:...] = sum_k wt_k.T @ x3[:, k, h*512:...]."""
            ps = [pp.tile([P, 512], f32, tag="mm", name=f"{nm}_h{h}")
                  for h in range(2)]
            if spot in FP8:
                K2 = KC // 2
                for i in range(K2):
                    w_ = wt[:, i, :, :]
                    for h in range(2):
                        nc.tensor.matmul(
                            ps[h][:], w_, x3[:, 2 * i:2 * i + 2,
                                             h * 512:(h + 1) * 512],
                            perf_mode=DR,
                            start=(i == 0), stop=(i == K2 - 1))
            else:
                for k in range(KC):
                    w_ = wt[:, k * P:(k + 1) * P]
                    for h in range(2):
                        nc.tensor.matmul(
                            ps[h][:], w_,
                            x3[:, k, h * 512:(h + 1) * 512],
                            start=(k == 0), stop=(k == KC - 1))
            for h in range(2):
                evict(h, ps[h])

        def wtile(wp, spot, src, KC, nm):
            if spot in FP8:
                t_ = wp.tile([P, KC // 2, 2, P], dt.float8e4, tag="w1",
                             name=nm)
            else:
                t_ = wp.tile([P, KC * P], b16, tag="w1", name=nm)
            nc.sync.dma_start(t_[:], src)
            return t_

        # ---------------- feed-forward macaron ----------------
        def ffn(tag, spot, w1d, c1tile, w2d, c2tile, d1, d2,
                stats_in, tail_hook):
            xdt = f8 if spot in FP8 else b16
            sx = SX if spot in FP8 else 1.0
            with ExitStack() as ctx:
                xq = xh_pool.tile([P, DC, T], xdt, tag="xq",
                                  name=f"xq_{tag}")
                stats_in.consume(xq, sx)
                wp = ctx.enter_context(tc.tile_pool(name=f"w_{tag}", bufs=3))
                hp = ctx.enter_context(tc.tile_pool(name=f"h1_{tag}", bufs=1))
                fv = ctx.enter_context(tc.tile_pool(name=f"fv_{tag}", bufs=4))
                h13 = hp.tile([P, FFC, T], xdt, tag="h13", name=f"h13_{tag}")
                with tc.tile_pool(name=f"ps1_{tag}", bufs=6,
                                  space="PSUM") as pp1:
                    for n in range(FFC):
                        wt = wtile(wp, spot, w1d[n], DC, f"w1_{tag}{n}")
                        if spot in FP8:
                            # silu evict -> bf16, then vector cast *SX -> fp8
                            hb_ = fv.tile([P, T], b16, tag="hb", bufs=4,
                                          name=f"hb_{tag}{n}")

                            def ev1(h, ps, hb_=hb_, n=n):
                                sl = slice(h * 512, (h + 1) * 512)
                                nc.scalar.activation(hb_[:, sl], ps[:],
                                                     Act.Silu,
                                                     bias=c1tile[:, n:n + 1],
                                                     scale=d1)
                            proj(pp1, wt, xq, DC, spot, f"p1_{tag}{n}", ev1)
                            nc.vector.tensor_scalar(h13[:, n, :], hb_[:], SX,
                                                    None, Alu.mult)
                        else:
                            def ev1(h, ps, n=n):
                                sl = slice(h * 512, (h + 1) * 512)
                                nc.scalar.activation(h13[:, n, sl], ps[:],
                                                     Act.Silu,
                                                     bias=c1tile[:, n:n + 1])
                            proj(pp1, wt, xq, DC, spot, f"p1_{tag}{n}", ev1)
                if DEBUG and tag == "ff1":
                    nc.sync.dma_start(ddram('d_h1', (P, T), xdt)[:],
                                      h13[:, 0, :])
                if tail_hook is not None:
                    tail_hook._open()
                pp = ctx.enter_context(
                    tc.tile_pool(name=f"ps2_{tag}", bufs=4, space="PSUM"))
                for dch in range(DC):
                    wt = wtile(wp, spot, w2d[dch], FFC, f"w2_{tag}{dch}")

                    def ev2(h, ps, dch=dch):
                        sl = slice(h * 512, (h + 1) * 512)
                        u = fv.tile([P, 512], f32, tag="fev", bufs=4,
                                    name=f"u2_{tag}{dch}_{h}")
                        nc.scalar.activation(u[:], ps[:], Act.Identity,
                                             bias=c2tile[:, dch:dch + 1],
                                             scale=d2)
                        nc.vector.tensor_tensor(
                            res[dch][:, sl], u[:],
                            res[dch].bitcast(f32)[:, sl], Alu.add)
                    proj(pp, wt, h13, FFC, spot, f"p2_{tag}{dch}", ev2)
                    if tail_hook is not None:
                        tail_hook.hook(dch)

        # ---------------- attention ----------------
        def attn(stats_in, tail_hook):
            with ExitStack() as ctx:
                xq_at = xh_pool.tile([P, DC, T],
                                     f8 if 'q' in FP8 else b16,
                                     tag="xq", name="xq_at")
                stats_in.consume(xq_at, SX if 'q' in FP8 else 1.0)
                wp = ctx.enter_context(tc.tile_pool(name="w_at", bufs=2))
                kv_pool = ctx.enter_context(tc.tile_pool(name="kvt", bufs=1))
                fv = ctx.enter_context(tc.tile_pool(name="fv_at", bufs=4))

                qpre, kva = [], []
                with tc.tile_pool(name="pA", bufs=4, space="PSUM") as pA, \
                        ExitStack() as lctx:
                    xq = xq_at
                    # kva projection first: k/v is the long dependency
                    # chain (latent LN -> kvb -> transpose/rope)
                    for n in range(RC):
                        wt = wtile(wp, 'kva', wkvaa[n], DC, f"wkva{n}")
                        kv_ = kv_pool.tile([P, T], b16, tag=f"kva{n}",
                                           name=f"kva{n}")

                        def evkva(h, ps, kv_=kv_, n=n):
                            sl = slice(h * 512, (h + 1) * 512)
                            nc.vector.tensor_scalar(
                                kv_[:, sl], ps[:], meta['d_wkvaa'],
                                ckvat[:, n:n + 1], Alu.mult, Alu.add)
                        proj(pA, wt, xq, DC, 'kva', f"pkva{n}", evkva)
                        kva.append(kv_)
                    if DEBUG:
                        dkva = ddram('d_kva', (R, T), b16)
                        nc.sync.dma_start(dkva[0:P, :], kva[0][:])
                        nc.sync.dma_start(dkva[P:R, :], kva[1][:])
                    # latent LN stats traced now (runs during q proj)
                    latdt = f8 if 'kvb' in FP8 else b16
                    lat3 = kv_pool.tile([P, RC, T], latdt, tag="lat3",
                                        name="lat3")
                    kvctx = ExitStack()
                    km, ka, klnp, krd = ln_stats(kvctx, "kv", kva, RC, R,
                                                 SX if 'kvb' in FP8 else 1.0)
                    # q projection -> qpre (bf16, pre-rope); fills the PE
                    # while the latent-stats vector chain completes
                    for n in range(8):
                        wt = wtile(wp, 'q', wqa[n], DC, f"wq{n}")
                        q_ = kv_pool.tile([P, T], b16, tag=f"q{n}",
                                          name=f"qpre{n}")

                        def evq(h, ps, q_=q_, n=n):
                            sl = slice(h * 512, (h + 1) * 512)
                            nc.scalar.activation(
                                q_[:, sl], ps[:], Act.Identity,
                                bias=cqt[:, n:n + 1], scale=meta['d_wqa'])
                        proj(pA, wt, xq, DC, 'q', f"pq{n}", evq)
                        qpre.append(q_)
                    if DEBUG:
                        nc.sync.dma_start(ddram('d_qp', (P, T), b16)[:],
                                          qpre[0][:])
                    ln_apply(kvctx, "kv", kva, RC, lat3, km, ka, klnp, krd)
                    kvctx.close()
                    if DEBUG:
                        dlat = ddram('d_lat', (R, T), latdt)
                        nc.sync.dma_start(dlat[0:P, :], lat3[:, 0, :])
                        nc.sync.dma_start(dlat[P:R, :], lat3[:, 1, :])
                    # kvb projection: rows 0..255 = k, 256..511 = v
                    kpre, vtt = [], []
                    for n in range(4):
                        wt = wtile(wp, 'kvb', wkvba[n], RC, f"wkvb{n}")
                        kv_ = kv_pool.tile([P, T], b16, tag=f"kvb{n}",
                                           name=f"kvb{n}")

                        def evkvb(h, ps, kv_=kv_, n=n):
                            sl = slice(h * 512, (h + 1) * 512)
                            nc.vector.tensor_scalar(
                                kv_[:, sl], ps[:], meta['d_wkvba'],
                                ckvbt[:, n:n + 1], Alu.mult, Alu.add)
                        proj(pA, wt, lat3, RC, 'kvb', f"pkvb{n}", evkvb)
                        (kpre if n < 2 else vtt).append(kv_)
                    if DEBUG:
                        dkv = ddram('d_kv', (R, T), b16)
                        nc.sync.dma_start(dkv[0:P, :], kpre[0][:])
                        nc.sync.dma_start(dkv[P:R, :], kpre[1][:])
                    # rope on k (first: feeds kr2 used by every head pair)
                    pR = lctx.enter_context(
                        tc.tile_pool(name="pR", bufs=2, space="PSUM"))

                    def rope_one(i, src, out_tag, pool=None, ptag="rope",
                                 obufs=1):
                        pq = (pool or pR).tile([P, T], f32, tag=ptag,
                                               name=f"ropep{i}")
                        for h in range(2):
                            sl = slice(h * 512, (h + 1) * 512)
                            nc.tensor.matmul(pq[:, sl], p2mt[:], src[:, sl],
                                             start=True, stop=True)
                        pqs = kv_pool.tile([P, T], b16, tag="pqs", bufs=2,
                                           name=f"pqs{i}")
                        nc.scalar.copy(pqs[:], pq[:])
                        t1 = kv_pool.tile([P, T], b16, tag="ropet1", bufs=2,
                                          name=f"ropet1_{i}")
                        nc.vector.tensor_tensor(t1[:], src[:], cos2t[:],
                                                Alu.mult)
                        t2 = kv_pool.tile([P, T], b16, tag="ropet2", bufs=2,
                                          name=f"ropet2_{i}")
                        nc.vector.tensor_tensor(t2[:], pqs[:], sin2t[:],
                                                Alu.mult)
                        r_ = kv_pool.tile([P, T], b16, tag=out_tag,
                                          bufs=obufs, name=f"roped{i}")
                        nc.vector.tensor_tensor(r_[:], t1[:], t2[:], Alu.add)
                        return r_

                    krc = [rope_one(8 + j, kpre[j], f"kro{j}")
                           for j in range(2)]
                    kr2 = []
                    for g_ in range(KVH):
                        k2 = kv_pool.tile([P, T], b16, tag=f"kr2_{g_}",
                                          name=f"kr2_{g_}")
                        off = 64 * (g_ % 2)
                        src = krc[g_ // 2]
                        nc.vector.tensor_copy(k2[0:64, :],
                                              src[off:off + 64, :])
                        nc.vector.tensor_copy(k2[64:P, :],
                                              src[off:off + 64, :])
                        kr2.append(k2)
                    # v: transpose to token-major + ones col -> vaug (bf16)
                    # merged [128,128] transposes cover two v-groups each
                    vaug = []
                    for g_ in range(KVH):
                        va = kv_pool.tile([P, DC, 65], b16, tag=f"va{g_}",
                                          name=f"vaug{g_}")
                        nc.gpsimd.memset(va[:, :, 64:65], 1.0)
                        vaug.append(va)
                    for vp in range(2):          # vtt[vp] holds groups 2vp,2vp+1
                        src = vtt[vp]
                        for c in range(DC):
                            pt_ = pA.tile([P, P], b16, tag="mm",
                                          name=f"vt{vp}_{c}")
                            nc.tensor.matmul(pt_[:],
                                             src[:, c * P:(c + 1) * P],
                                             identt[:],
                                             is_transpose=True,
                                             start=True, stop=True)
                            nc.scalar.copy(vaug[2 * vp][:, c, 0:64],
                                           pt_.bitcast(b16)[:, 0:64])
                            nc.scalar.copy(vaug[2 * vp + 1][:, c, 0:64],
                                           pt_.bitcast(b16)[:, 64:P])
                    # rope on q
                    qr = [rope_one(i, qpre[i], f"q{i}") for i in range(8)]
                if DEBUG:
                    nc.sync.dma_start(ddram('d_qr', (P, T), b16)[:], qr[0][:])
                    dkr = ddram('d_kr', (R, T), b16)
                    nc.sync.dma_start(dkr[0:P, :], krc[0][:])
                    nc.sync.dma_start(dkr[P:R, :], krc[1][:])

                # scores -> exp -> pT ; oT via vaug (denominator in row 64)
                odt = f8 if 'wo' in FP8 else b16
                ots3 = xh_pool.tile([P, DC, T], odt, tag="ots3", name="ots3")
                dden = ddram('d_den', (H, T), f32) if DEBUG else None
                with ExitStack() as sctx:
                    scp = sctx.enter_context(
                        tc.tile_pool(name="scp", bufs=2, space="PSUM"))
                    otp = sctx.enter_context(
                        tc.tile_pool(name="otp", bufs=1, space="PSUM"))
                    rbp = sctx.enter_context(
                        tc.tile_pool(name="rbp", bufs=1, space="PSUM"))
                    ptp = sctx.enter_context(tc.tile_pool(name="ptp", bufs=2))
                    otup = sctx.enter_context(tc.tile_pool(name="otup",
                                                           bufs=1))
                    for hp in range(8):
                        g_ = (2 * hp) // 4
                        kt = kr2[g_]
                        qt = qr[hp]
                        otu2 = []
                        for sub in range(2):
                            hh = 2 * hp + sub
                            qo = 64 * sub
                            pts = []
                            for c in range(DC):
                                sc = scp.tile([P, T], f32, tag="sc",
                                              name=f"sc{hh}_{c}")
                                for th in range(2):
                                    sl = slice(th * 512, (th + 1) * 512)
                                    nc.tensor.matmul(
                                        sc[:, sl],
                                        kt[qo:qo + 64, c * P:(c + 1) * P],
                                        qt[qo:qo + 64, sl],
                                        start=True, stop=True)
                                pt_ = ptp.tile([P, T], b16, tag=f"pt{c}",
                                               name=f"pt{hh}_{c}")
                                nc.scalar.activation(
                                    pt_[:], sc[:], Act.Exp,
                                    scale=float(HD) ** -0.5)
                                pts.append(pt_)
                            if DEBUG and hp == 0 and sub == 0:
                                nc.sync.dma_start(
                                    ddram('d_pt', (P, T), b16)[:], pts[0][:])
                            ou_ps = otp.tile([65, T], f32, tag="ou",
                                             name=f"oups{hh}")
                            for c in range(DC):
                                for th in range(2):
                                    sl = slice(th * 512, (th + 1) * 512)
                                    nc.tensor.matmul(
                                        ou_ps[:, sl], vaug[g_][:, c, :],
                                        pts[c][:, sl],
                                        start=(c == 0), stop=(c == DC - 1))
                            ou = otup.tile([65, T], f32, tag=f"otu{sub}",
                                           bufs=2, name=f"otu{hh}")
                            nc.vector.tensor_copy(ou[:], ou_ps[:])
                            otu2.append(ou)
                        # pair normalize (denominator sits in row 64)
                        den2 = otup.tile([2, T], f32, tag="den", bufs=1,
                                         name=f"den{hp}")
                        for sub in range(2):
                            nc.sync.dma_start(den2[sub:sub + 1, :],
                                              otu2[sub][64:65, :])
                        if DEBUG:
                            nc.sync.dma_start(dden[2 * hp:2 * hp + 2, :],
                                              den2[:])
                        recf2 = otup.tile([2, T], f32, tag="recf", bufs=1,
                                          name=f"recf{hp}")
                        nc.vector.reciprocal_approx_fast(out=recf2[:],
                                                         in_=den2[:])
                        recr2 = otup.tile([2, T], f32r, tag="recr",
                                          bufs=1, name=f"recr{hp}")
                        if 'wo' in FP8:
                            nc.vector.tensor_scalar(recr2[:], recf2[:], SO,
                                                    None, Alu.mult)
                        else:
                            nc.vector.tensor_copy(recr2[:], recf2[:])
                        rb = rbp.tile([P, T], f32, tag="rb", name=f"rb{hp}")
                        for th in range(2):
                            sl = slice(th * 512, (th + 1) * 512)
                            nc.tensor.matmul(rb[:, sl], sel2t[:],
                                             recr2[:, sl],
                                             start=True, stop=True)
                        for sub in range(2):
                            nc.vector.tensor_tensor(
                                ots3[sub * 64:(sub + 1) * 64, hp, :],
                                otu2[sub][0:64, :],
                                rb[sub * 64:(sub + 1) * 64, :], Alu.mult)
                if DEBUG:
                    nc.sync.dma_start(ddram('d_ot', (P, T), odt)[:],
                                      ots3[:, 0, :])
                # output projection + residual
                if tail_hook is not None:
                    tail_hook._open()
                with tc.tile_pool(name="pO", bufs=4, space="PSUM") as pO:
                    for dch in range(DC):
                        wt = wtile(wp, 'wo', woa[dch], DC, f"wo{dch}")

                        def evo(h, ps, dch=dch):
                            sl = slice(h * 512, (h + 1) * 512)
                            nc.vector.scalar_tensor_tensor(
                                res[dch][:, sl], ps[:], meta['d_woa'],
                                res[dch].bitcast(f32)[:, sl],
                                Alu.mult, Alu.add)
                        proj(pO, wt, ots3, DC, 'wo', f"po{dch}", evo)
                        if tail_hook is not None:
                            tail_hook.hook(dch)

        # ---------------- conv module ----------------
        def convmod(stats_in, tail_hook):
            with ExitStack() as ctx:
                xq = xh_pool.tile([P, DC, T], f8 if 'pw1' in FP8 else b16,
                                  tag="xq", name="xq_cv")
                stats_in.consume(xq, SX if 'pw1' in FP8 else 1.0)
                if tail_hook is not None:
                    tail_hook._open()
                wp = ctx.enter_context(tc.tile_pool(name="w_cv", bufs=3))
                ap_ = ctx.enter_context(tc.tile_pool(name="a_cv", bufs=1))
                fv = ctx.enter_context(tc.tile_pool(name="fv_cv", bufs=4))
                pp = ctx.enter_context(
                    tc.tile_pool(name="ps_cv", bufs=4, space="PSUM"))
                at, sg = [], []
                for n in range(16):
                    wt = wtile(wp, 'pw1', wp1a[n], DC, f"wp1_{n}")
                    o_ = ap_.tile([P, T], b16, tag=f"ag{n}", name=f"ag{n}")

                    def evc(h, ps, o_=o_, n=n):
                        sl = slice(h * 512, (h + 1) * 512)
                        nc.scalar.activation(
                            o_[:, sl], ps[:],
                            Act.Identity if n < 8 else Act.Sigmoid,
                            bias=cp1t[:, n:n + 1], scale=meta['d_wp1a'])
                    proj(pp, wt, xq, DC, 'pw1', f"pp1_{n}", evc)
                    (at if n < 8 else sg).append(o_)
                cvdt = f8 if 'pw2' in FP8 else b16
                cv3 = ap_.tile([P, DC, T], cvdt, tag="cv3", name="cv3")
                if 'dw' in FP8:
                    # glu8[c]: [P, 2, 1056] fp8*SX; copy1 = copy0 shifted 16
                    GW = 1056
                    glu8 = []
                    for c in range(DC):
                        gp = ap_.tile([P, 2, GW], f8, tag=f"glu{c}",
                                      name=f"glu8_{c}")
                        nc.gpsimd.memset(gp[:, 0, 0:15], 0.0)
                        nc.gpsimd.memset(gp[:, 0, T + 15:GW], 0.0)
                        nc.gpsimd.memset(gp[:, 1, T - 1:GW], 0.0)
                        nc.vector.scalar_tensor_tensor(
                            gp[:, 0, 15:T + 15], at[c][:], SX, sg[c][:],
                            Alu.mult, Alu.mult)
                        nc.vector.scalar_tensor_tensor(
                            gp[:, 1, 0:T - 1], at[c][:, 1:T], SX,
                            sg[c][:, 1:T], Alu.mult, Alu.mult)
                        glu8.append(gp)
                    if DEBUG:
                        dglu = ddram('d_glu8', (P, 2 * GW), f8)
                        nc.sync.dma_start(dglu[:, 0:GW], glu8[0][:, 0, :])
                        nc.sync.dma_start(dglu[:, GW:], glu8[0][:, 1, :])
                    for c in range(DC):
                        wt = wp.tile([P, 16, 2, P], f8, tag="diag", bufs=2,
                                     name=f"dg{c}")
                        nc.sync.dma_start(wt[:], dwdiag[c])
                        psc = [pp.tile([P, 512], f32, tag="mm",
                                       name=f"pcv{c}_{th}")
                               for th in range(2)]
                        for j in range(16):
                            for th in range(2):
                                o0 = th * 512 + j
                                nc.tensor.matmul(
                                    psc[th][:], wt[:, j, :, :],
                                    glu8[c][:, :, o0:o0 + 512],
                                    perf_mode=DR,
                                    start=(j == 0), stop=(j == 15))
                        for th in range(2):
                            sl = slice(th * 512, (th + 1) * 512)
                            nc.scalar.activation(
                                cv3[:, c, sl], psc[th][:], Act.Silu,
                                bias=tbnt[:, c:c + 1], scale=meta['d_dw'])
                else:
                    glu = []
                    for c in range(DC):
                        gp = ap_.tile([P, T + 30], b16, tag=f"glu{c}",
                                      name=f"glu{c}")
                        nc.gpsimd.memset(gp[:, 0:15], 0.0)
                        nc.gpsimd.memset(gp[:, T + 15:T + 30], 0.0)
                        nc.vector.tensor_tensor(gp[:, 15:T + 15], at[c][:],
                                                sg[c][:], Alu.mult)
                        glu.append(gp)
                    SPLIT = 21
                    for c in range(DC):
                        wt = wp.tile([P, SPLIT * P], b16, tag="diag", bufs=2,
                                     name=f"dg{c}")
                        nc.sync.dma_start(wt[:], dwdiag[c])
                        acc = ap_.tile([P, T], f32, tag="cacc", bufs=1,
                                       name=f"cacc{c}")
                        nc.vector.tensor_scalar(
                            acc[:], glu[c][:, SPLIT:SPLIT + T],
                            dwcolt[:, c * KW + SPLIT:c * KW + SPLIT + 1],
                            None, Alu.mult)
                        for j in range(SPLIT + 1, KW):
                            nc.vector.scalar_tensor_tensor(
                                acc[:], glu[c][:, j:j + T],
                                dwcolt[:, c * KW + j:c * KW + j + 1],
                                acc[:], Alu.mult, Alu.add)
                        psc = [pp.tile([P, 512], f32, tag="mm",
                                       name=f"pcv{c}_{th}")
                               for th in range(2)]
                        for j in range(SPLIT):
                            for th in range(2):
                                nc.tensor.matmul(
                                    psc[th][:], wt[:, j * P:(j + 1) * P],
                                    glu[c][:, th * 512 + j:th * 512 + j + 512],
                                    start=(j == 0), stop=(j == SPLIT - 1))
                        for th in range(2):
                            sl = slice(th * 512, (th + 1) * 512)
                            z_ = ap_.tile([P, 512], f32, tag="cz", bufs=1,
                                          name=f"cz{c}_{th}")
                            nc.vector.tensor_tensor(z_[:], acc[:, sl],
                                                    psc[th][:], Alu.add)
                            nc.scalar.activation(cv3[:, c, sl], z_[:],
                                                 Act.Silu,
                                                 bias=tbnt[:, c:c + 1])
                if DEBUG:
                    nc.sync.dma_start(ddram('d_cv', (P, T), cvdt)[:],
                                      cv3[:, 0, :])
                for dch in range(DC):
                    wt = wtile(wp, 'pw2', wp2a[dch], DC, f"wp2_{dch}")

                    def evp2(h, ps, dch=dch):
                        sl = slice(h * 512, (h + 1) * 512)
                        u = fv.tile([P, 512], f32, tag="fev", bufs=4,
                                    name=f"u_cv{dch}_{h}")
                        nc.scalar.activation(u[:], ps[:], Act.Identity,
                                             bias=cp2t[:, dch:dch + 1],
                                             scale=meta['d_wp2a'])
                        nc.vector.tensor_tensor(
                            res[dch][:, sl], u[:],
                            res[dch].bitcast(f32)[:, sl], Alu.add)
                    proj(pp, wt, cv3, DC, 'pw2', f"pp2_{dch}", evp2)
                    if tail_hook is not None:
                        tail_hook.hook(dch)

        # ---------------- final LN (with affine) ----------------
        def final_ln(stats_in):
            m_t, a_t = stats_in.finalize(1.0)
            mb, ab = stats_in.broadcast(m_t, a_t)
            outp = stats_in.ctx.enter_context(
                tc.tile_pool(name="outp", bufs=2))
            lns = stats_in.ctx.enter_context(
                tc.tile_pool(name="lns_fo", bufs=1))
            mbs = lns.tile([P, T], f32, name="mbs_fin")
            nc.scalar.copy(mbs[:], mb[:])
            for c in range(DC):
                tm = lns.tile([P, T], f32, tag="lntmp", bufs=3,
                              name=f"fintmp{c}")
                nc.gpsimd.tensor_tensor(tm[:], res[c].bitcast(f32),
                                        mbs[:], Alu.subtract)
                u_ = lns.tile([P, T], f32, tag="lnu", bufs=2,
                              name=f"finu{c}")
                nc.vector.scalar_tensor_tensor(u_[:], tm[:],
                                               fingt[:, c:c + 1], ab[:],
                                               Alu.mult, Alu.mult)
                o_ = outp.tile([P, T], f32r, tag="out", name=f"out{c}")
                nc.vector.tensor_scalar(o_[:], u_[:], finbt[:, c:c + 1],
                                        None, Alu.add)
                nc.sync.dma_start(outT[c * P:(c + 1) * P, :], o_[:])
            stats_in.ctx.close()

        # ---------------- phase sequencing ----------------
        ffn("ff1", 'ff1', w1a, c1t, w2a, c2t, meta['d_w1a'], meta['d_w2a'],
            st_ff1, st_at)
        if DEBUG:
            dr1 = ddram('d_res1', (D, T), f32r)
            for c in range(DC):
                nc.sync.dma_start(dr1[c * P:(c + 1) * P, :], res[c][:])
        if PHASES >= 2:
            attn(st_at, st_cv)
            if DEBUG:
                dr2 = ddram('d_res2', (D, T), f32r)
                for c in range(DC):
                    nc.sync.dma_start(dr2[c * P:(c + 1) * P, :], res[c][:])
        if PHASES >= 3:
            convmod(st_cv, st_f2)
            if DEBUG:
                dr3 = ddram('d_res3', (D, T), f32r)
                for c in range(DC):
                    nc.sync.dma_start(dr3[c * P:(c + 1) * P, :], res[c][:])
        if PHASES >= 4:
            ffn("ff2", 'ff2', w1b, c1bt, w2b, c2bt,
                meta['d_w1b'], meta['d_w2b'], st_f2, st_fin)
        if PHASES >= 5:
            final_ln(st_fin)
        else:
            for c in range(DC):
                nc.sync.dma_start(outT[c * P:(c + 1) * P, :], res[c][:])

    nc.compile()
    return nc


# ------------------------------------------------------------------ driver

_NC_CACHE = {}
meta = None  # set by prep_inputs; build() closes over it


def _get_nc(m):
    key = (PHASES, DEBUG, tuple(sorted(FP8)))
    if key not in _NC_CACHE:
        _NC_CACHE[key] = build(m)
    return _NC_CACHE[key]


def kernel(**inputs):
    global meta
    shared, m = prep_inputs(inputs)
    meta = m
    nc = _get_nc(m)
    x = np.asarray(inputs['x'], np.float32)
    in_maps = []
    for b in range(N_CORES):
        mm = dict(shared)
        mm['xT'] = np.ascontiguousarray(x[b].T)
        in_maps.append(mm)
    res = run_bass_kernel_spmd(nc, in_maps, core_ids=list(range(N_CORES)))
    out = np.stack([np.ascontiguousarray(r['outT'].T) for r in res.results])
    kernel.last_results = res
    return out.astype(np.float32)


# revision 72
# speedup vs baseline: 1.0288x; 1.0288x over previous
"""Conformer block (macaron FF + RMLA attention + gated depthwise conv) on
8 Trainium2 NeuronCores, data-parallel over batch (B=8 -> 1 seq/core).

The residual stream lives channel-major ([D, T]) in SBUF for the whole
kernel. LayerNorm is applied by standardizing the input once into a
quantized "xhat" tile (per-token mean/invstd come from ones-matmul
statistics + K=1 broadcast matmuls); gammas/betas are folded into the
adjacent weights/biases on the host. Projections run in fp8e4 with the
DoubleRow perf mode (2 k-chunks per matmul, 2x PE throughput); weights are
pre-scaled by a power of two into fp8's normal range and descaled for free
via the eviction activation's scale argument. The depthwise conv runs as
fp8 DoubleRow diagonal matmuls over shifted windows. Attention scores/probs
stay bf16 (softmax denominators via a ones-column appended to V).
"""
import os
from contextlib import ExitStack

import numpy as np
import ml_dtypes

import concourse.bacc as bacc
import concourse.tile as tile
import concourse.mybir as mybir
from concourse.bass_utils import run_bass_kernel_spmd

B, T, D = 8, 1024, 1024
H, HD, KVH, R = 16, 64, 4, 256
KW = 31
FF = 4 * D
EPS = 1e-5
P = 128
DC = D // P            # 8 residual chunks
FFC = FF // P          # 32
RC = R // P            # 2
N_CORES = 8

dt = mybir.dt
Alu = mybir.AluOpType
Act = mybir.ActivationFunctionType
DR = mybir.MatmulPerfMode.DoubleRow

bf16 = ml_dtypes.bfloat16
f8e4 = ml_dtypes.float8_e4m3fn

PHASES = int(os.environ.get("BASS_PHASES", "5"))
DEBUG = int(os.environ.get("BASS_DEBUG", "0"))
# fp8 spots (DoubleRow). ff1/ff2 toggle both matmuls of that FF block.
FP8 = set(os.environ.get(
    "BASS_FP8", "q,kva,kvb,wo,pw1,pw2,dw,ff1,ff2").split(","))

SX = 32.0              # activation pre-scale for fp8 xhat/h1/glu
SO = 8.0               # activation pre-scale for fp8 attention outputs


# ---------------------------------------------------------------- host prep

def _pow2_scale(w):
    m = float(np.abs(w).max())
    if m == 0.0:
        return 1.0
    return float(2.0 ** np.floor(np.log2(240.0 / m)))


def _shuffle_w(W):
    """[Kd, Nd] -> [NC, P, Kd] slab n: tile[:, kc*128:(kc+1)*128] =
    W[kc-chunk, n-chunk] (stationary for output chunk n, bf16 path)."""
    Kd, Nd = W.shape
    KC, NC = Kd // P, Nd // P
    arr = W.reshape(KC, P, NC, P).transpose(2, 1, 0, 3).reshape(NC, P, Kd)
    return np.ascontiguousarray(arr)


def _cols(v):
    """[N] bias -> [P, N/128] column tile (col n = bias of chunk n)."""
    return np.ascontiguousarray(v.reshape(-1, P).T)


def prep_inputs(inputs):
    f32 = np.float32
    g = {}
    meta = {}

    def W(name):
        return np.asarray(inputs[name], f32)

    def proj_w(key, wf, spot, sx):
        """Pack a [K, N] stationary; fp8 spots get DoubleRow layout
        [NC, P, KC//2, 2, P] with a pow2 scale; returns eviction descale."""
        if spot in FP8:
            sw = _pow2_scale(wf)
            arr = _shuffle_w(wf * sw)          # [NC, P, K]
            NC = arr.shape[0]
            KC = wf.shape[0] // P
            arr = arr.reshape(NC, P, KC // 2, 2, P)
            g[key] = np.ascontiguousarray(arr).astype(f8e4)
            return 1.0 / (sw * sx)
        g[key] = _shuffle_w(wf).astype(bf16)
        return 1.0

    # ff1 (LN gamma folded; 0.5 residual scale folded into w2/b2)
    ff1_on = 'ff1' in FP8
    meta['d_w1a'] = proj_w('w1a', W('ff1_ng')[:, None] * W('ff1_w1'),
                           'ff1', SX if ff1_on else 1.0)
    g['c1a'] = _cols(W('ff1_nb') @ W('ff1_w1') + W('ff1_b1'))
    meta['d_w2a'] = proj_w('w2a', 0.5 * W('ff1_w2'), 'ff1',
                           SX if ff1_on else 1.0)
    g['c2a'] = _cols(0.5 * W('ff1_b2'))
    # attention projections
    meta['d_wqa'] = proj_w('wqa', W('attn_ng')[:, None] * W('wq'), 'q', SX)
    g['cqa'] = _cols(W('attn_nb') @ W('wq'))
    meta['d_wkvaa'] = proj_w(
        'wkvaa', (W('attn_ng')[:, None] * W('wkva'))[:, :R], 'kva', SX)
    g['ckvaa'] = _cols((W('attn_nb') @ W('wkva'))[:R])
    meta['d_wkvba'] = proj_w('wkvba', W('kvn_g')[:, None] * W('wkvb'),
                             'kvb', SX)
    g['ckvba'] = _cols(W('kvn_b') @ W('wkvb'))
    meta['d_woa'] = proj_w('woa', W('wo'), 'wo', SO)
    # conv module
    meta['d_wp1a'] = proj_w('wp1a', W('conv_ng')[:, None] * W('pw1_w'),
                            'pw1', SX)
    g['cp1a'] = _cols(W('conv_nb') @ W('pw1_w') + W('pw1_b'))
    sbn = W('bn_g') / np.sqrt(W('bn_rv') + EPS)
    g['tbna'] = _cols((W('dw_b') - W('bn_rm')) * sbn + W('bn_b'))
    meta['d_wp2a'] = proj_w('wp2a', W('pw2_w'), 'pw2', 1.0)
    g['cp2a'] = _cols(W('pw2_b'))
    # depthwise conv as DoubleRow diag: pairs (tap j, tap j+16), j=0..15;
    # tap 31 zero-padded. dwdiag[c, p, j, t, col] = dwf[c*P+col, j+16*t]*(p==col)
    dwf = np.asarray(inputs['dw_w'], f32)[:, 0, :] * sbn[:, None]   # [D, 31]
    if 'dw' in FP8:
        sdw = _pow2_scale(dwf)
        dwp = np.concatenate([dwf * sdw, np.zeros((D, 1), f32)], 1)  # [D, 32]
        diag = np.zeros((DC, P, 16, 2, P), f32)
        idx = np.arange(P)
        for c in range(DC):
            for j in range(16):
                for t_ in range(2):
                    diag[c, idx, j, t_, idx] = dwp[c * P:(c + 1) * P, j + 16 * t_]
        g['dwdiag'] = diag.astype(f8e4)
        meta['d_dw'] = 1.0 / (sdw * SX)
    else:
        # bf16 fallback: 21 PE taps + 10 vector taps (baseline scheme)
        SPLIT = 21
        diag = np.zeros((DC, P, SPLIT, P), f32)
        idx = np.arange(P)
        for c in range(DC):
            for j in range(SPLIT):
                diag[c, idx, j, idx] = dwf[c * P:(c + 1) * P, j]
        g['dwdiag'] = diag.reshape(DC, P, SPLIT * P).astype(bf16)
        g['dwcol'] = np.ascontiguousarray(
            dwf.reshape(DC, P, KW).transpose(1, 0, 2).reshape(P, DC * KW))
        meta['d_dw'] = 1.0
    # ff2
    ff2_on = 'ff2' in FP8
    meta['d_w1b'] = proj_w('w1b', W('ff2_ng')[:, None] * W('ff2_w1'),
                           'ff2', SX if ff2_on else 1.0)
    g['c1b'] = _cols(W('ff2_nb') @ W('ff2_w1') + W('ff2_b1'))
    meta['d_w2b'] = proj_w('w2b', 0.5 * W('ff2_w2'), 'ff2',
                           SX if ff2_on else 1.0)
    g['c2b'] = _cols(0.5 * W('ff2_b2'))
    # final LN affine
    g['finga'] = _cols(W('fin_g'))
    g['finba'] = _cols(W('fin_b'))
    # rope tables (transposed, tiled x2 heads per 128 partitions)
    inv = 1.0 / (10000.0 ** (np.arange(0, HD, 2, dtype=f32) / HD))
    t = np.arange(T, dtype=f32)
    fr = np.einsum('i,j->ij', t, inv)
    emb = np.concatenate([fr, fr], -1)                        # [T, 64]
    cosT = np.cos(emb).T.astype(f32)                          # [64, T]
    sinT = np.sin(emb).T.astype(f32)
    g['cos2'] = np.ascontiguousarray(
        np.concatenate([cosT, cosT], 0)).astype(bf16)
    g['sin2'] = np.ascontiguousarray(
        np.concatenate([sinT, sinT], 0)).astype(bf16)
    p2 = np.zeros((P, P), f32)
    for b in range(2):
        o = 64 * b
        for d_ in range(32):
            p2[o + 32 + d_, o + d_] = -1.0
            p2[o + d_, o + 32 + d_] = 1.0
    g['p2m'] = p2.astype(bf16)
    g['ident'] = np.eye(P, dtype=f32).astype(bf16)
    g['ones1'] = np.ones((1, P), f32)
    sel2 = np.zeros((2, P), f32)
    sel2[0, 0:64] = 1.0
    sel2[1, 64:P] = 1.0
    g['sel2'] = sel2
    g['onesp'] = np.ones((P, 1), f32)
    g['onespb'] = np.ones((P, 1), f32).astype(bf16)
    return g, meta


# ------------------------------------------------------------- device build

def build(meta):
    nc = bacc.Bacc("TRN2", target_bir_lowering=False, debug=False,
                   enable_asserts=False, num_devices=N_CORES)
    f32, f32r, b16, f8 = dt.float32, dt.float32r, dt.bfloat16, dt.float8e4

    def wdt(spot):
        return f8 if spot in FP8 else b16

    def din(name, shape, d):
        return nc.dram_tensor(name, shape, d, kind="ExternalInput").ap()

    def wshape(spot, KC, NC):
        if spot in FP8:
            return (NC, P, KC // 2, 2, P)
        return (NC, P, KC * P)

    xT = din('xT', (D, T), f32r)
    w1a = din('w1a', wshape('ff1', DC, FFC), wdt('ff1'))
    c1a = din('c1a', (P, FFC), f32)
    w2a = din('w2a', wshape('ff1', FFC, DC), wdt('ff1'))
    c2a = din('c2a', (P, DC), f32)
    wqa = din('wqa', wshape('q', DC, 8), wdt('q'))
    cqa = din('cqa', (P, 8), f32)
    wkvaa = din('wkvaa', wshape('kva', DC, RC), wdt('kva'))
    ckvaa = din('ckvaa', (P, RC), f32)
    wkvba = din('wkvba', wshape('kvb', RC, 4), wdt('kvb'))
    ckvba = din('ckvba', (P, 4), f32)
    woa = din('woa', wshape('wo', DC, DC), wdt('wo'))
    wp1a = din('wp1a', wshape('pw1', DC, 16), wdt('pw1'))
    cp1a = din('cp1a', (P, 16), f32)
    tbna = din('tbna', (P, DC), f32)
    wp2a = din('wp2a', wshape('pw2', DC, DC), wdt('pw2'))
    cp2a = din('cp2a', (P, DC), f32)
    if 'dw' in FP8:
        dwdiag = din('dwdiag', (DC, P, 16, 2, P), f8)
        dwcold = None
    else:
        dwdiag = din('dwdiag', (DC, P, 21 * P), b16)
        dwcold = din('dwcol', (P, DC * KW), f32)
    w1b = din('w1b', wshape('ff2', DC, FFC), wdt('ff2'))
    c1b = din('c1b', (P, FFC), f32)
    w2b = din('w2b', wshape('ff2', FFC, DC), wdt('ff2'))
    c2b = din('c2b', (P, DC), f32)
    finga = din('finga', (P, DC), f32)
    finba = din('finba', (P, DC), f32)
    cos2d = din('cos2', (P, T), b16)
    sin2d = din('sin2', (P, T), b16)
    p2md = din('p2m', (P, P), b16)
    identd = din('ident', (P, P), b16)
    ones1d = din('ones1', (1, P), f32r)
    sel2d = din('sel2', (2, P), f32r)
    onespd = din('onesp', (P, 1), f32r)
    onespbd = din('onespb', (P, 1), b16)

    outT = nc.dram_tensor('outT', (D, T), f32r, kind="ExternalOutput").ap()

    def ddram(name, shape, d):
        return nc.dram_tensor(name, shape, d, kind="ExternalOutput").ap()

    with tile.TileContext(nc) as tc, ExitStack() as top:
        cpool = top.enter_context(tc.tile_pool(name="const", bufs=1))
        res_pool = top.enter_context(tc.tile_pool(name="res", bufs=1))
        xh_pool = top.enter_context(tc.tile_pool(name="xh", bufs=1))

        def ctile(src, shape, d, name):
            t_ = cpool.tile(shape, d, name=name)
            nc.sync.dma_start(t_[:], src[:])
            return t_

        c1t = ctile(c1a, [P, FFC], f32, "c1t")
        c2t = ctile(c2a, [P, DC], f32, "c2t")
        cqt = ctile(cqa, [P, 8], f32, "cqt")
        ckvat = ctile(ckvaa, [P, RC], f32, "ckvat")
        ckvbt = ctile(ckvba, [P, 4], f32, "ckvbt")
        cp1t = ctile(cp1a, [P, 16], f32, "cp1t")
        tbnt = ctile(tbna, [P, DC], f32, "tbnt")
        cp2t = ctile(cp2a, [P, DC], f32, "cp2t")
        c1bt = ctile(c1b, [P, FFC], f32, "c1bt")
        c2bt = ctile(c2b, [P, DC], f32, "c2bt")
        fingt = ctile(finga, [P, DC], f32, "fingt")
        finbt = ctile(finba, [P, DC], f32, "finbt")
        cos2t = ctile(cos2d, [P, T], b16, "cos2t")
        sin2t = ctile(sin2d, [P, T], b16, "sin2t")
        p2mt = ctile(p2md, [P, P], b16, "p2mt")
        identt = ctile(identd, [P, P], b16, "identt")
        ones1t = ctile(ones1d, [1, P], f32r, "ones1t")
        sel2t = ctile(sel2d, [2, P], f32r, "sel2t")
        onespt = ctile(onespd, [P, 1], f32r, "onespt")
        onespbt = ctile(onespbd, [P, 1], b16, "onespbt")
        dwcolt = (ctile(dwcold, [P, DC * KW], f32, "dwcolt")
                  if dwcold is not None else None)
        epst = cpool.tile([P, 1], dt.float32, name="epst")
        nc.gpsimd.memset(epst[:], EPS)

        # ------- cross-phase LN stats: accumulated in the producing -------
        # ------- phase's eviction tail, consumed at the next phase -------
        class NextStats:
            """Per-token LN stats over the residual, accumulated chunk-by-
            chunk as the previous phase finalizes each res chunk."""

            def __init__(self, tag, dred):
                self.tag, self.dred = tag, dred
                self.ctx = ExitStack()
                self.opened = False

            def _open(self):
                self.lnp = self.ctx.enter_context(
                    tc.tile_pool(name=f"lnp_{self.tag}", bufs=2,
                                 space="PSUM", side="right"))
                self.lns = self.ctx.enter_context(
                    tc.tile_pool(name=f"lns_{self.tag}", bufs=1,
                                 side="right"))
                self.s1 = self.lnp.tile([1, T], dt.float32, tag="lnps",
                                        name=f"s1_{self.tag}")
                self.s2 = self.lnp.tile([1, T], dt.float32, tag="lnps",
                                        name=f"s2_{self.tag}")
                self.opened = True

            def hook(self, c, nch=DC):
                assert self.opened, f"stats {self.tag} pools not opened"
                s_ = self.lns.tile([P, T], dt.float32r, tag="sq", bufs=2,
                                   name=f"sq_{self.tag}{c}")
                nc.scalar.square(s_[:], res[c].bitcast(dt.float32))
                for h in range(2):
                    sl = slice(h * 512, (h + 1) * 512)
                    nc.tensor.matmul(self.s1[:, sl], onespt[:],
                                     res[c][:, sl],
                                     start=(c == 0), stop=(c == nch - 1))
                for h in range(2):
                    sl = slice(h * 512, (h + 1) * 512)
                    nc.tensor.matmul(self.s2[:, sl], onespt[:], s_[:, sl],
                                     start=(c == 0), stop=(c == nch - 1))

            def finalize(self, sx):
                f32 = dt.float32
                tag = self.tag
                lns = self.lns
                m_t = lns.tile([1, T], dt.float32r, name=f"m_{tag}")
                a_t = lns.tile([1, T], dt.float32r, name=f"a_{tag}")
                nc.vector.tensor_scalar(m_t[:], self.s1[:], 1.0 / self.dred,
                                        None, Alu.mult)
                ms = lns.tile([1, T], f32, name=f"ms_{tag}")
                nc.vector.tensor_tensor(ms[:], m_t.bitcast(f32)[:],
                                        m_t.bitcast(f32)[:], Alu.mult)
                v_ = lns.tile([1, T], f32, name=f"v_{tag}")
                nc.vector.scalar_tensor_tensor(v_[:], self.s2[:],
                                               1.0 / self.dred, ms[:],
                                               Alu.mult, Alu.subtract)
                sd = lns.tile([1, T], f32, name=f"sd_{tag}")
                nc.scalar.activation(sd[:], v_[:], Act.Sqrt,
                                     bias=epst[0:1, 0:1])
                af = lns.tile([1, T], f32, name=f"af_{tag}")
                nc.vector.reciprocal_approx_fast(out=af[:], in_=sd[:])
                if sx != 1.0:
                    nc.vector.tensor_scalar(a_t[:], af[:], sx, None,
                                            Alu.mult)
                else:
                    nc.vector.tensor_copy(a_t[:], af[:])
                return m_t, a_t

            def broadcast(self, m_t, a_t):
                """Returns (mb_sbuf, ab_psum): gpsimd can't read PSUM."""
                f32 = dt.float32
                mb = self.lnp.tile([P, T], f32, tag="lnps",
                                   name=f"mb_{self.tag}")
                ab = self.lnp.tile([P, T], f32, tag="lnps",
                                   name=f"ab_{self.tag}")
                for h in range(2):
                    sl = slice(h * 512, (h + 1) * 512)
                    nc.tensor.matmul(mb[:, sl], ones1t[:], m_t[:, sl],
                                     start=True, stop=True)
                    nc.tensor.matmul(ab[:, sl], ones1t[:], a_t[:, sl],
                                     start=True, stop=True)
                return mb, ab

            def consume(self, dst3, sx, nch=DC):
                """Standardize res into dst3 (xhat pipeline: gpsimd sub,
                vector mult+cast); closes the stats pools. Must be called
                before the consuming phase opens any of its own pools."""
                m_t, a_t = self.finalize(sx)
                mb, ab = self.broadcast(m_t, a_t)
                f32 = dt.float32
                tm_p = self.ctx.enter_context(
                    tc.tile_pool(name=f"lntm_{self.tag}", bufs=3,
                                 side="right"))
                for c in range(nch):
                    tm = tm_p.tile([P, T], f32, tag="lntmp", bufs=3,
                                   name=f"lntmp_{self.tag}{c}")
                    nc.vector.tensor_tensor(tm[:], res[c].bitcast(f32),
                                            mb[:], Alu.subtract)
                    nc.vector.tensor_tensor(dst3[:, c, :], tm[:], ab[:],
                                            Alu.mult)
                self.ctx.close()

        st_ff1 = NextStats("ff1", D)
        st_at = NextStats("at", D) if PHASES >= 2 else None
        st_cv = NextStats("cv", D) if PHASES >= 3 else None
        st_f2 = NextStats("ff2", D) if PHASES >= 4 else None
        st_fin = NextStats("fin", D) if PHASES >= 5 else None

        st_ff1._open()
        res = []
        for c in range(DC):
            r_ = res_pool.tile([P, T], f32r, name=f"res{c}")
            nc.sync.dma_start(r_[:], xT[c * P:(c + 1) * P, :])
            res.append(r_)
        for c in range(DC):
            st_ff1.hook(c)

        # ------------- LN: stats + standardized (scaled) xhat -------------
        def ln_stats(ctx, tag, src_tiles, nch, dred, sx):
            """Per-token mean/scaled-invstd of src over nch*128 channels."""
            src_is_b16 = src_tiles[0].dtype == b16
            ones_stat = onespbt if src_is_b16 else onespt

            def rd(ap):
                return ap if src_is_b16 else ap.bitcast(f32)

            lnp = ctx.enter_context(
                tc.tile_pool(name=f"lnp_{tag}", bufs=2, space="PSUM",
                             side="right"))
            lns = ctx.enter_context(tc.tile_pool(name=f"lns_{tag}", bufs=1,
                                                 side="right"))
            sq = []
            for c in range(nch):
                s_ = lns.tile([P, T], f32r, tag="sq", bufs=2,
                              name=f"sq_{tag}{c}")
                nc.scalar.square(s_[:], rd(src_tiles[c][:]))
                sq.append(s_)
            s1 = lnp.tile([1, T], f32, tag="lnps", name=f"s1_{tag}")
            s2 = lnp.tile([1, T], f32, tag="lnps", name=f"s2_{tag}")
            for c in range(nch):
                for h in range(2):
                    sl = slice(h * 512, (h + 1) * 512)
                    nc.tensor.matmul(s1[:, sl], ones_stat[:],
                                     src_tiles[c][:, sl],
                                     start=(c == 0), stop=(c == nch - 1))
            for c in range(nch):
                for h in range(2):
                    sl = slice(h * 512, (h + 1) * 512)
                    nc.tensor.matmul(s2[:, sl], onespt[:], sq[c][:, sl],
                                     start=(c == 0), stop=(c == nch - 1))
            m_t = lns.tile([1, T], f32r, name=f"m_{tag}")
            a_t = lns.tile([1, T], f32r, name=f"a_{tag}")
            nc.vector.tensor_scalar(m_t[:], s1[:], 1.0 / dred, None, Alu.mult)
            ms = lns.tile([1, T], f32, name=f"ms_{tag}")
            nc.vector.tensor_tensor(ms[:], m_t.bitcast(f32)[:],
                                    m_t.bitcast(f32)[:], Alu.mult)
            v_ = lns.tile([1, T], f32, name=f"v_{tag}")
            nc.vector.scalar_tensor_tensor(v_[:], s2[:], 1.0 / dred, ms[:],
                                           Alu.mult, Alu.subtract)
            sd = lns.tile([1, T], f32, name=f"sd_{tag}")
            nc.scalar.activation(sd[:], v_[:], Act.Sqrt, bias=epst[0:1, 0:1])
            af = lns.tile([1, T], f32, name=f"af_{tag}")
            nc.vector.reciprocal_approx_fast(out=af[:], in_=sd[:])
            if sx != 1.0:
                nc.vector.tensor_scalar(a_t[:], af[:], sx, None, Alu.mult)
            else:
                nc.vector.tensor_copy(a_t[:], af[:])
            return m_t, a_t, lnp, rd

        def ln_apply(ctx, tag, src_tiles, nch, dst3, m_t, a_t, lnp, rd):
            """dst3[:, c, :] = (src_c - mean)*scaled_invstd per token."""
            mb = lnp.tile([P, T], f32, tag="lnps", name=f"mb_{tag}")
            ab = lnp.tile([P, T], f32, tag="lnps", name=f"ab_{tag}")
            for h in range(2):
                sl = slice(h * 512, (h + 1) * 512)
                nc.tensor.matmul(mb[:, sl], ones1t[:], m_t[:, sl],
                                 start=True, stop=True)
                nc.tensor.matmul(ab[:, sl], ones1t[:], a_t[:, sl],
                                 start=True, stop=True)
            tm_p = ctx.enter_context(tc.tile_pool(name=f"lntm_{tag}", bufs=2,
                                                  side="right"))
            for c in range(nch):
                tm = tm_p.tile([P, T], f32, tag="lntmp", bufs=2,
                               name=f"lntmp_{tag}{c}")
                nc.vector.tensor_tensor(tm[:], rd(src_tiles[c][:]), mb[:],
                                        Alu.subtract)
                nc.vector.tensor_tensor(dst3[:, c, :], tm[:], ab[:], Alu.mult)

        # ------------- projection: fp8 DoubleRow or bf16 -------------
        def proj(pp, wt, x3, KC, spot, nm, evict):
            """psum[:, h*512:...] = sum_k wt_k.T @ x3[:, k, h*512:...]."""
            ps = [pp.tile([P, 512], f32, tag="mm", name=f"{nm}_h{h}")
                  for h in range(2)]
            if spot in FP8:
                K2 = KC // 2
                for i in range(K2):
                    w_ = wt[:, i, :, :]
                    for h in range(2):
                        nc.tensor.matmul(
                            ps[h][:], w_, x3[:, 2 * i:2 * i + 2,
                                             h * 512:(h + 1) * 512],
                            perf_mode=DR,
                            start=(i == 0), stop=(i == K2 - 1))
            else:
                for k in range(KC):
                    w_ = wt[:, k * P:(k + 1) * P]
                    for h in range(2):
                        nc.tensor.matmul(
                            ps[h][:], w_,
                            x3[:, k, h * 512:(h + 1) * 512],
                            start=(k == 0), stop=(k == KC - 1))
            for h in range(2):
                evict(h, ps[h])

        def wtile(wp, spot, src, KC, nm):
            if spot in FP8:
                t_ = wp.tile([P, KC // 2, 2, P], dt.float8e4, tag="w1",
                             name=nm)
            else:
                t_ = wp.tile([P, KC * P], b16, tag="w1", name=nm)
            nc.sync.dma_start(t_[:], src)
            return t_

        # ---------------- feed-forward macaron ----------------
        def ffn(tag, spot, w1d, c1tile, w2d, c2tile, d1, d2,
                stats_in, tail_hook):
            xdt = f8 if spot in FP8 else b16
            sx = SX if spot in FP8 else 1.0
            with ExitStack() as ctx:
                xq = xh_pool.tile([P, DC, T], xdt, tag="xq",
                                  name=f"xq_{tag}")
                stats_in.consume(xq, sx)
                wp = ctx.enter_context(tc.tile_pool(name=f"w_{tag}", bufs=3))
                hp = ctx.enter_context(tc.tile_pool(name=f"h1_{tag}", bufs=1))
                fv = ctx.enter_context(tc.tile_pool(name=f"fv_{tag}", bufs=4))
                h13 = hp.tile([P, FFC, T], xdt, tag="h13", name=f"h13_{tag}")
                with tc.tile_pool(name=f"ps1_{tag}", bufs=6,
                                  space="PSUM") as pp1:
                    for n in range(FFC):
                        wt = wtile(wp, spot, w1d[n], DC, f"w1_{tag}{n}")
                        if spot in FP8:
                            # silu evict -> bf16, then vector cast *SX -> fp8
                            hb_ = fv.tile([P, T], b16, tag="hb", bufs=4,
                                          name=f"hb_{tag}{n}")

                            def ev1(h, ps, hb_=hb_, n=n):
                                sl = slice(h * 512, (h + 1) * 512)
                                nc.scalar.activation(hb_[:, sl], ps[:],
                                                     Act.Silu,
                                                     bias=c1tile[:, n:n + 1],
                                                     scale=d1)
                            proj(pp1, wt, xq, DC, spot, f"p1_{tag}{n}", ev1)
                            nc.vector.tensor_scalar(h13[:, n, :], hb_[:], SX,
                                                    None, Alu.mult)
                        else:
                            def ev1(h, ps, n=n):
                                sl = slice(h * 512, (h + 1) * 512)
                                nc.scalar.activation(h13[:, n, sl], ps[:],
                                                     Act.Silu,
                                                     bias=c1tile[:, n:n + 1])
                            proj(pp1, wt, xq, DC, spot, f"p1_{tag}{n}", ev1)
                if DEBUG and tag == "ff1":
                    nc.sync.dma_start(ddram('d_h1', (P, T), xdt)[:],
                                      h13[:, 0, :])
                if tail_hook is not None:
                    tail_hook._open()
                pp = ctx.enter_context(
                    tc.tile_pool(name=f"ps2_{tag}", bufs=4, space="PSUM"))
                for dch in range(DC):
                    wt = wtile(wp, spot, w2d[dch], FFC, f"w2_{tag}{dch}")

                    def ev2(h, ps, dch=dch):
                        sl = slice(h * 512, (h + 1) * 512)
                        u = fv.tile([P, 512], f32, tag="fev", bufs=4,
                                    name=f"u2_{tag}{dch}_{h}")
                        nc.scalar.activation(u[:], ps[:], Act.Identity,
                                             bias=c2tile[:, dch:dch + 1],
                                             scale=d2)
                        nc.vector.tensor_tensor(
                            res[dch][:, sl], u[:],
                            res[dch].bitcast(f32)[:, sl], Alu.add)
                    proj(pp, wt, h13, FFC, spot, f"p2_{tag}{dch}", ev2)
                    if tail_hook is not None:
                        tail_hook.hook(dch)

        # ---------------- attention ----------------
        def attn(stats_in, tail_hook):
            with ExitStack() as ctx:
                xq_at = xh_pool.tile([P, DC, T],
                                     f8 if 'q' in FP8 else b16,
                                     tag="xq", name="xq_at")
                stats_in.consume(xq_at, SX if 'q' in FP8 else 1.0)
                wp = ctx.enter_context(tc.tile_pool(name="w_at", bufs=2))
                kv_pool = ctx.enter_context(tc.tile_pool(name="kvt", bufs=1))
                fv = ctx.enter_context(tc.tile_pool(name="fv_at", bufs=4))

                qpre, kva = [], []
                with tc.tile_pool(name="pA", bufs=4, space="PSUM") as pA, \
                        ExitStack() as lctx:
                    xq = xq_at
                    # kva projection first: k/v is the long dependency
                    # chain (latent LN -> kvb -> transpose/rope)
                    for n in range(RC):
                        wt = wtile(wp, 'kva', wkvaa[n], DC, f"wkva{n}")
                        kv_ = kv_pool.tile([P, T], b16, tag=f"kva{n}",
                                           name=f"kva{n}")

                        def evkva(h, ps, kv_=kv_, n=n):
                            sl = slice(h * 512, (h + 1) * 512)
                            nc.vector.tensor_scalar(
                                kv_[:, sl], ps[:], meta['d_wkvaa'],
                                ckvat[:, n:n + 1], Alu.mult, Alu.add)
                        proj(pA, wt, xq, DC, 'kva', f"pkva{n}", evkva)
                        kva.append(kv_)
                    if DEBUG:
                        dkva = ddram('d_kva', (R, T), b16)
                        nc.sync.dma_start(dkva[0:P, :], kva[0][:])
                        nc.sync.dma_start(dkva[P:R, :], kva[1][:])
                    # latent LN stats traced now (runs during q proj)
                    latdt = f8 if 'kvb' in FP8 else b16
                    lat3 = kv_pool.tile([P, RC, T], latdt, tag="lat3",
                                        name="lat3")
                    kvctx = ExitStack()
                    km, ka, klnp, krd = ln_stats(kvctx, "kv", kva, RC, R,
                                                 SX if 'kvb' in FP8 else 1.0)
                    # q projection -> qpre (bf16, pre-rope); fills the PE
                    # while the latent-stats vector chain completes
                    for n in range(8):
                        wt = wtile(wp, 'q', wqa[n], DC, f"wq{n}")
                        q_ = kv_pool.tile([P, T], b16, tag=f"q{n}",
                                          name=f"qpre{n}")

                        def evq(h, ps, q_=q_, n=n):
                            sl = slice(h * 512, (h + 1) * 512)
                            nc.scalar.activation(
                                q_[:, sl], ps[:], Act.Identity,
                                bias=cqt[:, n:n + 1], scale=meta['d_wqa'])
                        proj(pA, wt, xq, DC, 'q', f"pq{n}", evq)
                        qpre.append(q_)
                    if DEBUG:
                        nc.sync.dma_start(ddram('d_qp', (P, T), b16)[:],
                                          qpre[0][:])
                    ln_apply(kvctx, "kv", kva, RC, lat3, km, ka, klnp, krd)
                    kvctx.close()
                    if DEBUG:
                        dlat = ddram('d_lat', (R, T), latdt)
                        nc.sync.dma_start(dlat[0:P, :], lat3[:, 0, :])
                        nc.sync.dma_start(dlat[P:R, :], lat3[:, 1, :])
                    # kvb projection: rows 0..255 = k, 256..511 = v
                    kpre, vtt = [], []
                    for n in range(4):
                        wt = wtile(wp, 'kvb', wkvba[n], RC, f"wkvb{n}")
                        kv_ = kv_pool.tile([P, T], b16, tag=f"kvb{n}",
                                           name=f"kvb{n}")

                        def evkvb(h, ps, kv_=kv_, n=n):
                            sl = slice(h * 512, (h + 1) * 512)
                            nc.vector.tensor_scalar(
                                kv_[:, sl], ps[:], meta['d_wkvba'],
                                ckvbt[:, n:n + 1], Alu.mult, Alu.add)
                        proj(pA, wt, lat3, RC, 'kvb', f"pkvb{n}", evkvb)
                        (kpre if n < 2 else vtt).append(kv_)
                    if DEBUG:
                        dkv = ddram('d_kv', (R, T), b16)
                        nc.sync.dma_start(dkv[0:P, :], kpre[0][:])
                        nc.sync.dma_start(dkv[P:R, :], kpre[1][:])
                    # rope on k (first: feeds kr2 used by every head pair)
                    pR = lctx.enter_context(
                        tc.tile_pool(name="pR", bufs=2, space="PSUM"))

                    def rope_one(i, src, out_tag, pool=None, ptag="rope",
                                 obufs=1):
                        pq = (pool or pR).tile([P, T], f32, tag=ptag,
                                               name=f"ropep{i}")
                        for h in range(2):
                            sl = slice(h * 512, (h + 1) * 512)
                            nc.tensor.matmul(pq[:, sl], p2mt[:], src[:, sl],
                                             start=True, stop=True)
                        pqs = kv_pool.tile([P, T], b16, tag="pqs", bufs=2,
                                           name=f"pqs{i}")
                        nc.scalar.copy(pqs[:], pq[:])
                        t1 = kv_pool.tile([P, T], b16, tag="ropet1", bufs=2,
                                          name=f"ropet1_{i}")
                        nc.vector.tensor_tensor(t1[:], src[:], cos2t[:],
                                                Alu.mult)
                        t2 = kv_pool.tile([P, T], b16, tag="ropet2", bufs=2,
                                          name=f"ropet2_{i}")
                        nc.vector.tensor_tensor(t2[:], pqs[:], sin2t[:],
                                                Alu.mult)
                        r_ = kv_pool.tile([P, T], b16, tag=out_tag,
                                          bufs=obufs, name=f"roped{i}")
                        nc.vector.tensor_tensor(r_[:], t1[:], t2[:], Alu.add)
                        return r_

                    krc = [rope_one(8 + j, kpre[j], f"kro{j}")
                           for j in range(2)]
                    kr2 = []
                    for g_ in range(KVH):
                        k2 = kv_pool.tile([P, T], b16, tag=f"kr2_{g_}",
                                          name=f"kr2_{g_}")
                        off = 64 * (g_ % 2)
                        src = krc[g_ // 2]
                        nc.vector.tensor_copy(k2[0:64, :],
                                              src[off:off + 64, :])
                        nc.vector.tensor_copy(k2[64:P, :],
                                              src[off:off + 64, :])
                        kr2.append(k2)
                    # v: transpose to token-major + ones col -> vaug (bf16)
                    # merged [128,128] transposes cover two v-groups each
                    vaug = []
                    for g_ in range(KVH):
                        va = kv_pool.tile([P, DC, 65], b16, tag=f"va{g_}",
                                          name=f"vaug{g_}")
                        nc.gpsimd.memset(va[:, :, 64:65], 1.0)
                        vaug.append(va)
                    for vp in range(2):          # vtt[vp] holds groups 2vp,2vp+1
                        src = vtt[vp]
                        for c in range(DC):
                            pt_ = pA.tile([P, P], b16, tag="mm",
                                          name=f"vt{vp}_{c}")
                            nc.tensor.matmul(pt_[:],
                                             src[:, c * P:(c + 1) * P],
                                             identt[:],
                                             is_transpose=True,
                                             start=True, stop=True)
                            nc.scalar.copy(vaug[2 * vp][:, c, 0:64],
                                           pt_.bitcast(b16)[:, 0:64])
                            nc.scalar.copy(vaug[2 * vp + 1][:, c, 0:64],
                                           pt_.bitcast(b16)[:, 64:P])
                    # rope on q
                    qr = [rope_one(i, qpre[i], f"q{i}") for i in range(8)]
                if DEBUG:
                    nc.sync.dma_start(ddram('d_qr', (P, T), b16)[:], qr[0][:])
                    dkr = ddram('d_kr', (R, T), b16)
                    nc.sync.dma_start(dkr[0:P, :], krc[0][:])
                    nc.sync.dma_start(dkr[P:R, :], krc[1][:])

                # scores -> exp -> pT ; oT via vaug (denominator in row 64)
                odt = f8 if 'wo' in FP8 else b16
                ots3 = xh_pool.tile([P, DC, T], odt, tag="ots3", name="ots3")
                dden = ddram('d_den', (H, T), f32) if DEBUG else None
                with ExitStack() as sctx:
                    scp = sctx.enter_context(
                        tc.tile_pool(name="scp", bufs=2, space="PSUM"))
                    otp = sctx.enter_context(
                        tc.tile_pool(name="otp", bufs=1, space="PSUM"))
                    rbp = sctx.enter_context(
                        tc.tile_pool(name="rbp", bufs=1, space="PSUM"))
                    ptp = sctx.enter_context(tc.tile_pool(name="ptp", bufs=2))
                    otup = sctx.enter_context(tc.tile_pool(name="otup",
                                                           bufs=1))
                    for hp in range(8):
                        g_ = (2 * hp) // 4
                        kt = kr2[g_]
                        qt = qr[hp]
                        otu2 = []
                        for sub in range(2):
                            hh = 2 * hp + sub
                            qo = 64 * sub
                            pts = []
                            for c in range(DC):
                                sc = scp.tile([P, T], f32, tag="sc",
                                              name=f"sc{hh}_{c}")
                                for th in range(2):
                                    sl = slice(th * 512, (th + 1) * 512)
                                    nc.tensor.matmul(
                                        sc[:, sl],
                                        kt[qo:qo + 64, c * P:(c + 1) * P],
                                        qt[qo:qo + 64, sl],
                                        start=True, stop=True)
                                pt_ = ptp.tile([P, T], b16, tag=f"pt{c}",
                                               name=f"pt{hh}_{c}")
                                nc.scalar.activation(
                                    pt_[:], sc[:], Act.Exp,
                                    scale=float(HD) ** -0.5)
                                pts.append(pt_)
                            if DEBUG and hp == 0 and sub == 0:
                                nc.sync.dma_start(
                                    ddram('d_pt', (P, T), b16)[:], pts[0][:])
                            ou_ps = otp.tile([65, T], f32, tag="ou",
                                             name=f"oups{hh}")
                            for c in range(DC):
                                for th in range(2):
                                    sl = slice(th * 512, (th + 1) * 512)
                                    nc.tensor.matmul(
                                        ou_ps[:, sl], vaug[g_][:, c, :],
                                        pts[c][:, sl],
                                        start=(c == 0), stop=(c == DC - 1))
                            ou = otup.tile([65, T], f32, tag=f"otu{sub}",
                                           bufs=2, name=f"otu{hh}")
                            nc.vector.tensor_copy(ou[:], ou_ps[:])
                            otu2.append(ou)
                        # pair normalize (denominator sits in row 64)
                        den2 = otup.tile([2, T], f32, tag="den", bufs=1,
                                         name=f"den{hp}")
                        for sub in range(2):
                            nc.sync.dma_start(den2[sub:sub + 1, :],
                                              otu2[sub][64:65, :])
                        if DEBUG:
                            nc.sync.dma_start(dden[2 * hp:2 * hp + 2, :],
                                              den2[:])
                        recf2 = otup.tile([2, T], f32, tag="recf", bufs=1,
                                          name=f"recf{hp}")
                        nc.vector.reciprocal_approx_fast(out=recf2[:],
                                                         in_=den2[:])
                        recr2 = otup.tile([2, T], f32r, tag="recr",
                                          bufs=1, name=f"recr{hp}")
                        if 'wo' in FP8:
                            nc.vector.tensor_scalar(recr2[:], recf2[:], SO,
                                                    None, Alu.mult)
                        else:
                            nc.vector.tensor_copy(recr2[:], recf2[:])
                        rb = rbp.tile([P, T], f32, tag="rb", name=f"rb{hp}")
                        for th in range(2):
                            sl = slice(th * 512, (th + 1) * 512)
                            nc.tensor.matmul(rb[:, sl], sel2t[:],
                                             recr2[:, sl],
                                             start=True, stop=True)
                        for sub in range(2):
                            nc.vector.tensor_tensor(
                                ots3[sub * 64:(sub + 1) * 64, hp, :],
                                otu2[sub][0:64, :],
                                rb[sub * 64:(sub + 1) * 64, :], Alu.mult)
                if DEBUG:
                    nc.sync.dma_start(ddram('d_ot', (P, T), odt)[:],
                                      ots3[:, 0, :])
                # output projection + residual
                if tail_hook is not None:
                    tail_hook._open()
                with tc.tile_pool(name="pO", bufs=4, space="PSUM") as pO:
                    for dch in range(DC):
                        wt = wtile(wp, 'wo', woa[dch], DC, f"wo{dch}")

                        def evo(h, ps, dch=dch):
                            sl = slice(h * 512, (h + 1) * 512)
                            nc.vector.scalar_tensor_tensor(
                                res[dch][:, sl], ps[:], meta['d_woa'],
                                res[dch].bitcast(f32)[:, sl],
                                Alu.mult, Alu.add)
                        proj(pO, wt, ots3, DC, 'wo', f"po{dch}", evo)
                        if tail_hook is not None:
                            tail_hook.hook(dch)

        # ---------------- conv module ----------------
        def convmod(stats_in, tail_hook):
            with ExitStack() as ctx:
                xq = xh_pool.tile([P, DC, T], f8 if 'pw1' in FP8 else b16,
                                  tag="xq", name="xq_cv")
                stats_in.consume(xq, SX if 'pw1' in FP8 else 1.0)
                if tail_hook is not None:
                    tail_hook._open()
                wp = ctx.enter_context(tc.tile_pool(name="w_cv", bufs=3))
                ap_ = ctx.enter_context(tc.tile_pool(name="a_cv", bufs=1))
                fv = ctx.enter_context(tc.tile_pool(name="fv_cv", bufs=4))
                pp = ctx.enter_context(
                    tc.tile_pool(name="ps_cv", bufs=4, space="PSUM"))
                at, sg = [], []
                for n in range(16):
                    wt = wtile(wp, 'pw1', wp1a[n], DC, f"wp1_{n}")
                    o_ = ap_.tile([P, T], b16, tag=f"ag{n}", name=f"ag{n}")

                    def evc(h, ps, o_=o_, n=n):
                        sl = slice(h * 512, (h + 1) * 512)
                        nc.scalar.activation(
                            o_[:, sl], ps[:],
                            Act.Identity if n < 8 else Act.Sigmoid,
                            bias=cp1t[:, n:n + 1], scale=meta['d_wp1a'])
                    proj(pp, wt, xq, DC, 'pw1', f"pp1_{n}", evc)
                    (at if n < 8 else sg).append(o_)
                cvdt = f8 if 'pw2' in FP8 else b16
                cv3 = ap_.tile([P, DC, T], cvdt, tag="cv3", name="cv3")
                if 'dw' in FP8:
                    # glu8[c]: [P, 2, 1056] fp8*SX; copy1 = copy0 shifted 16
                    GW = 1056
                    glu8 = []
                    for c in range(DC):
                        gp = ap_.tile([P, 2, GW], f8, tag=f"glu{c}",
                                      name=f"glu8_{c}")
                        nc.gpsimd.memset(gp[:, 0, 0:15], 0.0)
                        nc.gpsimd.memset(gp[:, 0, T + 15:GW], 0.0)
                        nc.gpsimd.memset(gp[:, 1, T - 1:GW], 0.0)
                        nc.vector.scalar_tensor_tensor(
                            gp[:, 0, 15:T + 15], at[c][:], SX, sg[c][:],
                            Alu.mult, Alu.mult)
                        nc.vector.scalar_tensor_tensor(
                            gp[:, 1, 0:T - 1], at[c][:, 1:T], SX,
                            sg[c][:, 1:T], Alu.mult, Alu.mult)
                        glu8.append(gp)
                    if DEBUG:
                        dglu = ddram('d_glu8', (P, 2 * GW), f8)
                        nc.sync.dma_start(dglu[:, 0:GW], glu8[0][:, 0, :])
                        nc.sync.dma_start(dglu[:, GW:], glu8[0][:, 1, :])
                    for c in range(DC):
                        wt = wp.tile([P, 16, 2, P], f8, tag="diag", bufs=2,
                                     name=f"dg{c}")
                        nc.sync.dma_start(wt[:], dwdiag[c])
                        psc = [pp.tile([P, 512], f32, tag="mm",
                                       name=f"pcv{c}_{th}")
                               for th in range(2)]
                        for j in range(16):
                            for th in range(2):
                                o0 = th * 512 + j
                                nc.tensor.matmul(
                                    psc[th][:], wt[:, j, :, :],
                                    glu8[c][:, :, o0:o0 + 512],
                                    perf_mode=DR,
                                    start=(j == 0), stop=(j == 15))
                        for th in range(2):
                            sl = slice(th * 512, (th + 1) * 512)
                            nc.scalar.activation(
                                cv3[:, c, sl], psc[th][:], Act.Silu,
                                bias=tbnt[:, c:c + 1], scale=meta['d_dw'])
                else:
                    glu = []
                    for c in range(DC):
                        gp = ap_.tile([P, T + 30], b16, tag=f"glu{c}",
                                      name=f"glu{c}")
                        nc.gpsimd.memset(gp[:, 0:15], 0.0)
                        nc.gpsimd.memset(gp[:, T + 15:T + 30], 0.0)
                        nc.vector.tensor_tensor(gp[:, 15:T + 15], at[c][:],
                                                sg[c][:], Alu.mult)
                        glu.append(gp)
                    SPLIT = 21
                    for c in range(DC):
                        wt = wp.tile([P, SPLIT * P], b16, tag="diag", bufs=2,
                                     name=f"dg{c}")
                        nc.sync.dma_start(wt[:], dwdiag[c])
                        acc = ap_.tile([P, T], f32, tag="cacc", bufs=1,
                                       name=f"cacc{c}")
                        nc.vector.tensor_scalar(
                            acc[:], glu[c][:, SPLIT:SPLIT + T],
                            dwcolt[:, c * KW + SPLIT:c * KW + SPLIT + 1],
                            None, Alu.mult)
                        for j in range(SPLIT + 1, KW):
                            nc.vector.scalar_tensor_tensor(
                                acc[:], glu[c][:, j:j + T],
                                dwcolt[:, c * KW + j:c * KW + j + 1],
                                acc[:], Alu.mult, Alu.add)
                        psc = [pp.tile([P, 512], f32, tag="mm",
                                       name=f"pcv{c}_{th}")
                               for th in range(2)]
                        for j in range(SPLIT):
                            for th in range(2):
                                nc.tensor.matmul(
                                    psc[th][:], wt[:, j * P:(j + 1) * P],
                                    glu[c][:, th * 512 + j:th * 512 + j + 512],
                                    start=(j == 0), stop=(j == SPLIT - 1))
                        for th in range(2):
                            sl = slice(th * 512, (th + 1) * 512)
                            z_ = ap_.tile([P, 512], f32, tag="cz", bufs=1,
                                          name=f"cz{c}_{th}")
                            nc.vector.tensor_tensor(z_[:], acc[:, sl],
                                                    psc[th][:], Alu.add)
                            nc.scalar.activation(cv3[:, c, sl], z_[:],
                                                 Act.Silu,
                                                 bias=tbnt[:, c:c + 1])
                if DEBUG:
                    nc.sync.dma_start(ddram('d_cv', (P, T), cvdt)[:],
                                      cv3[:, 0, :])
                for dch in range(DC):
                    wt = wtile(wp, 'pw2', wp2a[dch], DC, f"wp2_{dch}")

                    def evp2(h, ps, dch=dch):
                        sl = slice(h * 512, (h + 1) * 512)
                        u = fv.tile([P, 512], f32, tag="fev", bufs=4,
                                    name=f"u_cv{dch}_{h}")
                        nc.scalar.activation(u[:], ps[:], Act.Identity,
                                             bias=cp2t[:, dch:dch + 1],
                                             scale=meta['d_wp2a'])
                        nc.vector.tensor_tensor(
                            res[dch][:, sl], u[:],
                            res[dch].bitcast(f32)[:, sl], Alu.add)
                    proj(pp, wt, cv3, DC, 'pw2', f"pp2_{dch}", evp2)
                    if tail_hook is not None:
                        tail_hook.hook(dch)

        # ---------------- final LN (with affine) ----------------
        def final_ln(stats_in):
            m_t, a_t = stats_in.finalize(1.0)
            mb, ab = stats_in.broadcast(m_t, a_t)
            outp = stats_in.ctx.enter_context(
                tc.tile_pool(name="outp", bufs=2))
            lns = stats_in.ctx.enter_context(
                tc.tile_pool(name="lns_fo", bufs=1))
            mbs = lns.tile([P, T], f32, name="mbs_fin")
            nc.scalar.copy(mbs[:], mb[:])
            for c in range(DC):
                tm = lns.tile([P, T], f32, tag="lntmp", bufs=3,
                              name=f"fintmp{c}")
                nc.gpsimd.tensor_tensor(tm[:], res[c].bitcast(f32),
                                        mbs[:], Alu.subtract)
                u_ = lns.tile([P, T], f32, tag="lnu", bufs=2,
                              name=f"finu{c}")
                nc.vector.scalar_tensor_tensor(u_[:], tm[:],
                                               fingt[:, c:c + 1], ab[:],
                                               Alu.mult, Alu.mult)
                o_ = outp.tile([P, T], f32r, tag="out", name=f"out{c}")
                nc.vector.tensor_scalar(o_[:], u_[:], finbt[:, c:c + 1],
                                        None, Alu.add)
                nc.sync.dma_start(outT[c * P:(c + 1) * P, :], o_[:])
            stats_in.ctx.close()

        # ---------------- phase sequencing ----------------
        ffn("ff1", 'ff1', w1a, c1t, w2a, c2t, meta['d_w1a'], meta['d_w2a'],
            st_ff1, st_at)
        if DEBUG:
            dr1 = ddram('d_res1', (D, T), f32r)
            for c in range(DC):
                nc.sync.dma_start(dr1[c * P:(c + 1) * P, :], res[c][:])
        if PHASES >= 2:
            attn(st_at, st_cv)
            if DEBUG:
                dr2 = ddram('d_res2', (D, T), f32r)
                for c in range(DC):
                    nc.sync.dma_start(dr2[c * P:(c + 1) * P, :], res[c][:])
        if PHASES >= 3:
            convmod(st_cv, st_f2)
            if DEBUG:
                dr3 = ddram('d_res3', (D, T), f32r)
                for c in range(DC):
                    nc.sync.dma_start(dr3[c * P:(c + 1) * P, :], res[c][:])
        if PHASES >= 4:
            ffn("ff2", 'ff2', w1b, c1bt, w2b, c2bt,
                meta['d_w1b'], meta['d_w2b'], st_f2, st_fin)
        if PHASES >= 5:
            final_ln(st_fin)
        else:
            for c in range(DC):
                nc.sync.dma_start(outT[c * P:(c + 1) * P, :], res[c][:])

    nc.compile()
    return nc


# ------------------------------------------------------------------ driver

_NC_CACHE = {}
meta = None  # set by prep_inputs; build() closes over it


def _get_nc(m):
    key = (PHASES, DEBUG, tuple(sorted(FP8)))
    if key not in _NC_CACHE:
        _NC_CACHE[key] = build(m)
    return _NC_CACHE[key]


def kernel(**inputs):
    global meta
    shared, m = prep_inputs(inputs)
    meta = m
    nc = _get_nc(m)
    x = np.asarray(inputs['x'], np.float32)
    in_maps = []
    for b in range(N_CORES):
        mm = dict(shared)
        mm['xT'] = np.ascontiguousarray(x[b].T)
        in_maps.append(mm)
    res = run_bass_kernel_spmd(nc, in_maps, core_ids=list(range(N_CORES)))
    out = np.stack([np.ascontiguousarray(r['outT'].T) for r in res.results])
    kernel.last_results = res
    return out.astype(np.float32)
